# revision 9
# baseline (speedup 1.0000x reference)
"""Trainium2 Bass kernel for MultiHead GQA attention (B=2, S=2048, D=2048,
H=16 query heads, HKV=4 kv heads, DH=128, RoPE, mask, out-proj).

Sharding: token-parallel across 8 cores. Core c handles batch c//4 and 512
query rows of it. Each core projects K/V for its own 512-token quarter
(all 4 kv heads), the quarters are all-gathered in 4 pipelined 128-token
chunks, and the core runs attention + out-proj for its rows. Host
reassembles. All matmuls bf16 with fp32 PSUM accumulation.

Causal handling (exact, SPMD-uniform): core r of its batch owns the 16
interleaved 32-row q-blocks {4j + r : j=0..15} (ascending). For key tile
kc the q-blocks that attend to it are exactly the suffix of blocks with
position j >= kc, i.e. a contiguous column suffix of width n = 32*(16-kc)
-- identical on every core. Only the first 32 columns of each suffix (the
diagonal block) are partially masked; they get multiplied by a per-core
[128, 32] 0/1 tile. This computes 34 128x128-tile-equivalents per head
(the exact causal minimum for a 4-way row split) vs 40 for the previous
128-row-block scheme.

Attention is computed transposed: scoresT[keys, q] = khT.T @ qhT per
128-key tile, exp on ScalarE (scale folded in), probs bf16, then
outT[dh, q] += v_tile.T @ probsT, and row-sums via a ones-stationary
matmul. outT feeds the out-projection directly as stationary operand.

The K/V all-gather is split into 4 collectives, one per 128-token block
of each rank's quarter: chunk m delivers key tiles {4r + m : r=0..3}.
Attention iterates kc in the order [0,4,8,12, 1,5,9,13, ...] so the
first kc group only needs chunk 0 -- the remaining chunks stream in
behind attention/Q-proj compute instead of serializing in front of it.

Mask modes (host-detected, compile-time): none / causal / mask as before;
"mask" computes the full rectangle (n=512) and multiplies by the 0/1 mask.
"""

import math

import numpy as np
import ml_dtypes

import concourse.bass as bass
import concourse.mybir as mybir
import concourse.tile as tile
from concourse import bacc
from concourse.bass_utils import run_bass_kernel_spmd

F32 = mybir.dt.float32
BF16 = mybir.dt.bfloat16
BF = ml_dtypes.bfloat16

B, S, D = 2, 2048, 2048
H, G = 16, 4
HKV = H // G            # 4
DH = D // H             # 128
DKV = D // G            # 512 (kv projection width)
NCORES = 8
RPC = S // 4            # 512 rows per core
NIC = D // 128          # 16 contraction chunks
NKC = S // 128          # 16 key tiles
SCALE = 1.0 / math.sqrt(DH)
# attention kc order: the K/V all-gather is split in 2 chunks; chunk A
# carries 128-token blocks {0,1} of each rank's quarter (= key tiles
# {4r, 4r+1}), chunk B blocks {2,3}.  Attention processes all A tiles
# first so it can start as soon as chunk A lands.
KC_ORDER = [4 * b + m for b in range(4) for m in range(2)] + \
           [4 * b + m for b in range(4) for m in range(2, 4)]

_NC_CACHE: dict = {}

# set by callers (e.g. test.py) to capture a profile; results of the last run
TRACE = False
TRACE_CORES = None          # e.g. [0] or list(range(8))
LAST_RESULTS = None


def _n_list(mode: str) -> list[int]:
    """Moving-operand width (in q columns, suffix of the 512) per key tile."""
    if mode == "causal":
        return [32 * (16 - kc) for kc in range(NKC)]
    return [512] * NKC


def _build(mode: str):
    mask_mul = mode != "none"
    n_of = _n_list(mode)

    nc = bacc.Bacc("TRN2", target_bir_lowering=False, debug=False,
                   num_devices=NCORES)

    # ---- I/O (host-prepared layouts; all contiguous-DMA friendly) ----
    wq = nc.declare_dram_parameter("wq", [NIC, 128, D], BF16, isOutput=False)
    qt = nc.declare_dram_parameter("qt", [128, NIC * RPC], BF16, isOutput=False)
    # k/v: only this core's 512-token quarter (projected here, all-gathered)
    kt = nc.declare_dram_parameter("kt", [128, NIC * 512], BF16, isOutput=False)
    vt = nc.declare_dram_parameter("vt", [4, 128, NIC * 128], BF16, isOutput=False)
    wk = nc.declare_dram_parameter("wk", [HKV, 128, NIC * 128], BF16, isOutput=False)
    wv = nc.declare_dram_parameter("wv", [128, NIC * DKV], BF16, isOutput=False)
    wo = nc.declare_dram_parameter("wo", [4, 128, H * 512], BF16, isOutput=False)
    cosq = nc.declare_dram_parameter("cosq", [128, RPC], BF16, isOutput=False)
    sinq = nc.declare_dram_parameter("sinq", [128, RPC], BF16, isOutput=False)
    # cos/sin for this core's own k-token quarter
    cosk = nc.declare_dram_parameter("cosk", [128, 512], BF16, isOutput=False)
    sink = nc.declare_dram_parameter("sink", [128, 512], BF16, isOutput=False)
    pswap = nc.declare_dram_parameter("pswap", [128, 128], BF16, isOutput=False)
    if mode == "causal":
        mdiag = nc.declare_dram_parameter("mdiag", [128, 32], BF16,
                                          isOutput=False)
    if mode == "mask":
        m01 = nc.declare_dram_parameter("m01", [128, NKC * RPC], BF16,
                                        isOutput=False)
    out = nc.declare_dram_parameter("out", [RPC, D], F32, isOutput=True)

    with tile.TileContext(nc) as tc:
        with (
            tc.tile_pool(name="res", bufs=1) as res,          # resident
            tc.tile_pool(name="stream2m", bufs=2) as stream2m,  # 2MB blocks
            tc.tile_pool(name="stream05", bufs=3) as stream05,  # 0.5MB blocks
            tc.tile_pool(name="small", bufs=3) as small,
            tc.tile_pool(name="probs", bufs=8) as probsp,
            tc.tile_pool(name="bcast", bufs=2) as bcastp,
            tc.tile_pool(name="dram", bufs=1, space="DRAM") as dramp,
            tc.tile_pool(name="psmm", bufs=5, space="PSUM") as psmm,
            tc.tile_pool(name="psacc", bufs=2, space="PSUM") as psacc,
            tc.tile_pool(name="pssum", bufs=1, space="PSUM") as pssum,
        ):
            # ---------------- resident tiles (DMAs staged per phase) -------
            # K-path first so the first matmul isn't stuck behind bulk loads
            coskq_t = res.tile([128, 512], BF16)
            nc.sync.dma_start(out=coskq_t, in_=cosk[:, :])
            sinkq_t = res.tile([128, 512], BF16)
            nc.sync.dma_start(out=sinkq_t, in_=sink[:, :])
            pswap_t = res.tile([128, 128], BF16)
            nc.sync.dma_start(out=pswap_t, in_=pswap[:, :])
            ones_t = res.tile([128, 1], BF16)
            nc.vector.memset(ones_t, 1.0)
            if mode == "causal":
                mdiag_t = res.tile([128, 32], BF16)
                nc.sync.dma_start(out=mdiag_t, in_=mdiag[:, :])
            # allocated here (tag order: qts before outu_a), loaded later
            qts = res.tile([128, NIC, RPC], BF16)

            qhs = res.tile([128, H, RPC], BF16)     # rope'd q, [dh, h, rows]
            khs = res.tile([128, HKV, S], BF16)     # rope'd k, [dh, hk, keys]
            vhs = res.tile([128, 16, DKV], BF16)    # v heads, [tok%128, tokc, kv]
            # outu_a shares qts's slot: qts is dead once phase A finishes.
            # split 12/4 so phase D's early matmuls (h<12) don't dep-chain
            # behind the last normalization batch (h>=12).
            outu_a = res.tile([128, 12, RPC], BF16, tag="qts")
            outu_b = res.tile([128, 4, RPC], BF16)

            def outu(h):
                return outu_a[:, h, :] if h < 12 else outu_b[:, h - 12, :]
            # normalization batches: heads [0:8], [8:12], [12:16]
            NB = [(0, 8), (8, 12), (12, 16)]
            sums_g = [res.tile([8, RPC], F32, name=f"sums{g}", tag=f"sums{g}")
                      for g in range(len(NB))]
            rec_g = [res.tile([8, RPC], F32, name=f"rec{g}", tag=f"rec{g}")
                     for g in range(len(NB))]
            sums_dram = dramp.tile([16, RPC], F32)
            rec_dram = dramp.tile([16, RPC], F32)
            khs_own = res.tile([128, HKV, 512], BF16)
            vhs_own = res.tile([128, 4, DKV], BF16)
            # 2-chunk staging: chunk m = own 128-token blocks {2m, 2m+1}
            # [128, 0:1024] = K (4 hk x 256 tok), [128, 1024:2048] = V
            kv_cin = dramp.tile([2, 128, 2048], BF16)
            # [chunk, rank, 128, 2048] so each chunk's gather output is
            # contiguous
            kv_cout = dramp.tile([2, 4, 128, 2048], BF16)

            def rope(dst, x_bf, ps_pool, cos_ap, sin_ap, n):
                """dst = x*cos + pairswap(x)*sin  (signs baked into sin)."""
                y_ps = ps_pool.tile([128, 512], F32, tag="mm")
                # moving operand max 1024 bf16 per matmul
                assert n <= 512
                nc.tensor.matmul(y_ps[:, :n], pswap_t, x_bf, start=True,
                                 stop=True)
                t1 = small.tile([128, 512], BF16, tag="t1")
                nc.vector.tensor_mul(t1[:, :n], x_bf, cos_ap)
                t2 = small.tile([128, 512], BF16, tag="t2")
                nc.vector.tensor_mul(t2[:, :n], y_ps[:, :n], sin_ap)
                nc.vector.tensor_add(dst, t1[:, :n], t2[:, :n])

            # ------- Phase B: K/V proj for OWN 512-token quarter + RoPE -----
            # (first, so the chunked all-gather overlaps Q proj + attention)
            kmov = stream2m.tile([128, NIC, 512], BF16, tag="s2m")
            # split the load so the first matmul starts ASAP
            for icq in range(4):
                nc.sync.dma_start(
                    out=kmov[:, 4 * icq:4 * icq + 4, :],
                    in_=kt[:, 4 * icq * 512:(4 * icq + 4) * 512].rearrange(
                        "p (i m) -> p i m", i=4))
            for hk in range(HKV):
                wk_all = stream05.tile([128, NIC, 128], BF16, tag="s05")
                nc.sync.dma_start(out=wk_all, in_=wk[hk].rearrange(
                    "p (i m) -> p i m", i=NIC))
                ps = psmm.tile([128, 512], F32, tag="mm")
                for ic in range(NIC):
                    nc.tensor.matmul(ps, wk_all[:, ic, :],
                                     kmov[:, ic, :],
                                     start=(ic == 0), stop=(ic == NIC - 1))
                xk = small.tile([128, 512], BF16, tag="xq")
                nc.scalar.copy(xk, ps)
                rope(khs_own[:, hk, :], xk, psmm, coskq_t, sinkq_t, 512)

            wvs = res.tile([128, NIC, DKV], BF16)
            nc.sync.dma_start(out=wvs, in_=wv[:, :].rearrange(
                "p (i n) -> p i n", i=NIC))
            for j in range(4):            # own 128-token blocks (V stationary)
                vmov = stream05.tile([128, NIC, 128], BF16, tag="s05")
                nc.sync.dma_start(out=vmov, in_=vt[j].rearrange(
                    "p (i m) -> p i m", i=NIC))
                ps = psmm.tile([128, 512], F32, tag="mm")
                for ic in range(NIC):
                    nc.tensor.matmul(ps, vmov[:, ic, :],
                                     wvs[:, ic, :],
                                     start=(ic == 0), stop=(ic == NIC - 1))
                nc.vector.tensor_copy(vhs_own[:, j, :], ps)
                if j % 2 == 0:
                    continue
                # stage + all-gather chunk m = blocks {2m, 2m+1}
                m = j // 2
                nc.sync.dma_start(
                    out=kv_cin[m, :, 0:1024].rearrange("p (h m) -> p h m",
                                                       h=HKV),
                    in_=khs_own[:, :, 256 * m:256 * (m + 1)])
                nc.sync.dma_start(
                    out=kv_cin[m, :, 1024:2048].rearrange("p (v m) -> p v m",
                                                          v=2),
                    in_=vhs_own[:, 2 * m:2 * m + 2, :])
                nc.gpsimd.collective_compute(
                    "AllGather", mybir.AluOpType.bypass,
                    replica_groups=[[0, 1, 2, 3], [4, 5, 6, 7]],
                    ins=[kv_cin[m]], outs=[kv_cout[m]])
                # unstage: chunk m of rank r covers key tiles {4r+2m, 4r+2m+1}
                for r in range(4):
                    nc.sync.dma_start(
                        out=khs[:, :, 512 * r + 256 * m:512 * r + 256 * (m + 1)],
                        in_=kv_cout[m, r, :, 0:1024].rearrange(
                            "p (h x) -> p h x", h=HKV))
                    nc.sync.dma_start(
                        out=vhs[:, 4 * r + 2 * m:4 * r + 2 * m + 2, :],
                        in_=kv_cout[m, r, :, 1024:2048].rearrange(
                            "p (v x) -> p v x", v=2))

            # ---------------- Phase A: Q-proj + RoPE ----------------
            nc.sync.dma_start(out=qts, in_=qt[:, :].rearrange(
                "p (i m) -> p i m", i=NIC))
            cosq_t = res.tile([128, RPC], BF16)
            nc.sync.dma_start(out=cosq_t, in_=cosq[:, :])
            sinq_t = res.tile([128, RPC], BF16)
            nc.sync.dma_start(out=sinq_t, in_=sinq[:, :])
            for oc in range(H):
                wq_all = stream05.tile([128, NIC, 128], BF16, tag="s05")
                nc.sync.dma_start(out=wq_all, in_=wq[oc].rearrange(
                    "p (i m) -> p i m", i=NIC))
                ps = psmm.tile([128, 512], F32, tag="mm")
                for ic in range(NIC):
                    nc.tensor.matmul(ps, wq_all[:, ic, :],
                                     qts[:, ic, :],
                                     start=(ic == 0), stop=(ic == NIC - 1))
                xq = small.tile([128, 512], BF16, tag="xq")
                nc.scalar.copy(xq, ps)
                rope(qhs[:, oc, :], xq, psmm, cosq_t, sinq_t, RPC)

            # ---------------- Phase C: attention per head ----------------
            if mode == "mask":
                m01s = res.tile([128, NKC, RPC], BF16)
                nc.sync.dma_start(out=m01s, in_=m01[:, :].rearrange(
                    "p (k m) -> p k m", k=NKC))

            def normalize_batch(g):
                """reciprocal + broadcast + in-place normalize for the heads
                of batch g (their sums are already in sums_dram)."""
                a, bnd = NB[g]
                m = bnd - a
                nc.sync.dma_start(out=sums_g[g][:m, :],
                                  in_=sums_dram[a:bnd, :])
                nc.vector.reciprocal(rec_g[g][:m, :], sums_g[g][:m, :])
                nc.sync.dma_start(out=rec_dram[a:bnd, :], in_=rec_g[g][:m, :])
                for h in range(a, bnd):
                    recb = bcastp.tile([128, RPC], F32, tag="bc")
                    nc.sync.dma_start(
                        out=recb,
                        in_=rec_dram[h:h + 1, :].to_broadcast([128, RPC]))
                    nc.vector.tensor_mul(outu(h), outu(h), recb)

            LA = 2                # scores/exp lookahead (software pipeline)

            for h in range(H):
                hk = h // G
                ps_o = psacc.tile([128, 512], F32, tag="acc")
                ps_s = pssum.tile([1, 512], F32, tag="sum")
                pending = {}

                def issue_scores(i, h=h, hk=hk, pending=pending):
                    kc = KC_ORDER[i]
                    n = n_of[kc]
                    lo = RPC - n          # suffix columns
                    ps_sc = psmm.tile([128, 512], F32, tag="mm")
                    nc.tensor.matmul(
                        ps_sc[:, :n],
                        khs[:, hk, kc * 128:(kc + 1) * 128],
                        qhs[:, h, lo:],
                        start=True, stop=True, skip_group_check=True)
                    probs = probsp.tile([128, 512], BF16, tag="pr")
                    nc.scalar.activation(
                        probs[:, :n], ps_sc[:, :n],
                        mybir.ActivationFunctionType.Exp, scale=SCALE)
                    if mode == "causal":
                        # only the first 32 suffix columns (the diagonal
                        # 32-row q-block) are partially masked
                        nc.vector.tensor_mul(probs[:, :32], probs[:, :32],
                                             mdiag_t)
                    elif mask_mul:
                        nc.vector.tensor_mul(probs[:, :n], probs[:, :n],
                                             m01s[:, kc, lo:])
                    pending[i] = (probs, kc, n, lo)

                for i in range(LA):
                    issue_scores(i)
                for idx in range(NKC):
                    if idx + LA < NKC:
                        issue_scores(idx + LA)
                    probs, kc, n, lo = pending.pop(idx)
                    first = idx == 0
                    last = idx == NKC - 1
                    nc.tensor.matmul(ps_s[:, lo:], ones_t, probs[:, :n],
                                     start=first, stop=last,
                                     skip_group_check=True)
                    nc.tensor.matmul(
                        ps_o[:, lo:],
                        vhs[:, kc, hk * 128:(hk + 1) * 128],
                        probs[:, :n],
                        start=first, stop=last, skip_group_check=True)
                sm1 = small.tile([1, RPC], F32, tag="sm1", bufs=2)
                nc.vector.tensor_copy(sm1, ps_s)
                nc.sync.dma_start(out=sums_dram[h:h + 1, :], in_=sm1)
                nc.vector.tensor_copy(outu(h), ps_o)
                if h == 7:
                    normalize_batch(0)
                elif h == 11:
                    normalize_batch(1)
            normalize_batch(2)

            # ---------------- Phase D: out-projection ----------------
            for oc in range(4):
                wo_all = stream2m.tile([128, H, 512], BF16, tag="s2m")
                nc.sync.dma_start(out=wo_all, in_=wo[oc].rearrange(
                    "p (h m) -> p h m", h=H))
                for qc in range(4):
                    ps_f = psmm.tile([128, 512], F32, tag="mm")
                    for h in range(H):
                        lh = outu_a[:, h, qc * 128:(qc + 1) * 128] if h < 12 \
                            else outu_b[:, h - 12, qc * 128:(qc + 1) * 128]
                        nc.tensor.matmul(
                            ps_f, lh, wo_all[:, h, :],
                            start=(h == 0), stop=(h == H - 1))
                    fin = small.tile([128, 512], F32, tag="fin")
                    nc.vector.tensor_copy(fin, ps_f)
                    nc.sync.dma_start(
                        out=out[qc * 128:(qc + 1) * 128,
                                oc * 512:(oc + 1) * 512],
                        in_=fin)

    nc.compile()
    return nc


def _get_nc(mode: str):
    if mode not in _NC_CACHE:
        _NC_CACHE[mode] = _build(mode)
    return _NC_CACHE[mode]


def _core_rows(mode: str, r: int) -> np.ndarray:
    """Global (within-batch) q-row indices owned by quarter r, ascending.

    causal: 16 interleaved 32-row blocks {4j + r : j} -> exact suffix
    causality, identical shapes on every core.  other modes: contiguous.
    """
    if mode == "causal":
        return np.concatenate([np.arange(32 * (4 * j + r), 32 * (4 * j + r + 1))
                               for j in range(16)])
    return np.arange(r * RPC, (r + 1) * RPC)


def kernel(q, k, v, mask, freqs, W_q, W_k, W_v, W_o):
    q = np.asarray(q, dtype=np.float32)
    k = np.asarray(k, dtype=np.float32)
    v = np.asarray(v, dtype=np.float32)
    mask = np.asarray(mask, dtype=np.float32)
    freqs = np.asarray(freqs, dtype=np.float32)
    W_q = np.asarray(W_q, dtype=np.float32)
    W_k = np.asarray(W_k, dtype=np.float32)
    W_v = np.asarray(W_v, dtype=np.float32)
    W_o = np.asarray(W_o, dtype=np.float32)

    # ---- mask mode detection ----
    nz = mask != 0
    if nz.all():
        mode = "none"
    else:
        tril = np.tril(np.ones((S, S), dtype=bool))
        mode = "causal" if all(np.array_equal(nz[b], tril) for b in range(B)) \
            else "mask"

    # ---- shared host precomputation ----
    c_full = np.cos(freqs)                      # [S, 64]
    s_full = np.sin(freqs)
    sgn = np.tile(np.array([-1.0, 1.0], np.float32), DH // 2)  # [-,+,-,+...]
    cosk_h = np.repeat(c_full, 2, axis=1).T.astype(BF)          # [128, S]
    sink_h = (np.repeat(s_full, 2, axis=1) * sgn).T.astype(BF)

    psw = np.zeros((128, 128), np.float32)
    idx = np.arange(128)
    psw[idx, idx ^ 1] = 1.0
    psw = psw.astype(BF)

    # weight layouts
    # wq[oc, p, i*128+m] = W_q[oc*128+m, i*128+p]
    wq_h = np.ascontiguousarray(
        W_q.reshape(H, 128, NIC, 128).transpose(0, 3, 2, 1)
        .reshape(H, 128, D)).astype(BF)
    # wk[hk, p, i*128+m] = W_k[hk*128+m, i*128+p]
    wk_h = np.ascontiguousarray(
        W_k.reshape(HKV, 128, NIC, 128).transpose(0, 3, 2, 1)
        .reshape(HKV, 128, D)).astype(BF)
    # wv[p, i*512+n] = W_v[n, i*128+p]
    wv_h = np.ascontiguousarray(
        W_v.reshape(DKV, NIC, 128).transpose(2, 1, 0).reshape(128, NIC * DKV)
    ).astype(BF)
    # wo[oc, p, h*512+m] = W_o[oc*512+m, h*128+p]
    wo_h = np.ascontiguousarray(
        W_o.reshape(4, 512, H, 128).transpose(0, 3, 2, 1).reshape(4, 128, -1)
    ).astype(BF)

    # k/v: each core only gets its own 512-token quarter (gathered on device)
    # kt[p, i*512+t] = k[b, tq*512+t, i*128+p] for quarter tq
    kt_b = []   # [B][4] quarters
    vt_b = []
    for b in range(B):
        kt_b.append([np.ascontiguousarray(
            k[b, tq * 512:(tq + 1) * 512].reshape(512, NIC, 128)
            .transpose(2, 1, 0).reshape(128, NIC * 512)).astype(BF)
            for tq in range(4)])
        # vt[j, p, i*128+t] = v[b, tq*512 + j*128+t, i*128+p]
        vt_b.append([np.ascontiguousarray(
            v[b, tq * 512:(tq + 1) * 512].reshape(4, 128, NIC, 128)
            .transpose(0, 3, 2, 1).reshape(4, 128, NIC * 128)).astype(BF)
            for tq in range(4)])

    in_maps = []
    rows_all = []
    for c in range(NCORES):
        b, r = divmod(c, 4)
        rows = _core_rows(mode, r)
        rows_all.append((b, rows))
        # qt[p, i*512+t] = q[b, rows[t], i*128+p]
        qsl = q[b][rows]                       # [512, D]
        qt_h = np.ascontiguousarray(
            qsl.reshape(RPC, NIC, 128).transpose(2, 1, 0).reshape(128, -1)
        ).astype(BF)
        cq = np.repeat(c_full[rows], 2, axis=1).T.astype(BF)      # [128, 512]
        sq = (np.repeat(s_full[rows], 2, axis=1) * sgn).T.astype(BF)
        im = {
            "wq": wq_h, "qt": qt_h, "kt": kt_b[b][r], "vt": vt_b[b][r],
            "wk": wk_h, "wv": wv_h, "wo": wo_h,
            "cosq": cq, "sinq": sq,
            "cosk": np.ascontiguousarray(cosk_h[:, r * 512:(r + 1) * 512]),
            "sink": np.ascontiguousarray(sink_h[:, r * 512:(r + 1) * 512]),
            "pswap": psw,
        }
        if mode == "causal":
            # diagonal 32-col block mask: keep key p of tile kc for local
            # row i of block j=kc  <=>  p <= 32*r + i  (same for all kc)
            pp = np.arange(128)[:, None]
            ii = np.arange(32)[None, :]
            im["mdiag"] = (pp <= 32 * r + ii).astype(BF)
        elif mode == "mask":
            # m01[p, kc*512+m] = (mask[b, rows[m], kc*128+p] != 0)
            msl = nz[b][rows]                  # [512, S] bool
            m01_h = np.ascontiguousarray(
                msl.T.reshape(NKC, 128, RPC).transpose(1, 0, 2)
                .reshape(128, -1)).astype(BF)
            im["m01"] = m01_h
        in_maps.append(im)

    nc = _get_nc(mode)
    kwargs = {}
    if TRACE:
        kwargs["trace"] = True
        if TRACE_CORES:
            kwargs["trace_cores"] = list(TRACE_CORES)
    results = run_bass_kernel_spmd(nc, in_maps, core_ids=list(range(NCORES)),
                                   **kwargs)
    global LAST_RESULTS
    LAST_RESULTS = results

    full = np.empty((B, S, D), np.float32)
    for c in range(NCORES):
        b, rows = rows_all[c]
        full[b, rows] = results.results[c]["out"]
    return full


# revision 13
# speedup vs baseline: 1.0828x; 1.0828x over previous
"""Trainium2 Bass kernel for MultiHead GQA attention (B=2, S=2048, D=2048,
H=16 query heads, HKV=4 kv heads, DH=128, RoPE, mask, out-proj).

Sharding: token-parallel across 8 cores. Core c handles batch c//4 and 512
query rows of it. Each core projects K/V for its own 512-token quarter
(all 4 kv heads), the quarters are all-gathered in 4 pipelined 128-token
chunks, and the core runs attention + out-proj for its rows. Host
reassembles. All matmuls bf16 with fp32 PSUM accumulation.

Causal handling (exact, SPMD-uniform): core r of its batch owns the 16
interleaved 32-row q-blocks {4j + r : j=0..15} (ascending). For key tile
kc the q-blocks that attend to it are exactly the suffix of blocks with
position j >= kc, i.e. a contiguous column suffix of width n = 32*(16-kc)
-- identical on every core. Only the first 32 columns of each suffix (the
diagonal block) are partially masked; they get multiplied by a per-core
[128, 32] 0/1 tile. This computes 34 128x128-tile-equivalents per head
(the exact causal minimum for a 4-way row split) vs 40 for the previous
128-row-block scheme.

Attention is computed transposed: scoresT[keys, q] = khT.T @ qhT per
128-key tile, exp on ScalarE (scale folded in), probs bf16, then
outT[dh, q] += v_tile.T @ probsT, and row-sums via a ones-stationary
matmul. outT feeds the out-projection directly as stationary operand.

The K/V all-gather is split into 4 collectives, one per 128-token block
of each rank's quarter: chunk m delivers key tiles {4r + m : r=0..3}.
Attention iterates kc in the order [0,4,8,12, 1,5,9,13, ...] so the
first kc group only needs chunk 0 -- the remaining chunks stream in
behind attention/Q-proj compute instead of serializing in front of it.

Mask modes (host-detected, compile-time): none / causal / mask as before;
"mask" computes the full rectangle (n=512) and multiplies by the 0/1 mask.
"""

import math

import numpy as np
import ml_dtypes

import concourse.bass as bass
import concourse.mybir as mybir
import concourse.tile as tile
from concourse import bacc
from concourse.bass_utils import run_bass_kernel_spmd

F32 = mybir.dt.float32
BF16 = mybir.dt.bfloat16
BF = ml_dtypes.bfloat16

B, S, D = 2, 2048, 2048
H, G = 16, 4
HKV = H // G            # 4
DH = D // H             # 128
DKV = D // G            # 512 (kv projection width)
NCORES = 8
RPC = S // 4            # 512 rows per core
NIC = D // 128          # 16 contraction chunks
NKC = S // 128          # 16 key tiles
SCALE = 1.0 / math.sqrt(DH)
# attention kc order: the K/V all-gather is split in 2 chunks; chunk A
# carries 128-token blocks {0,1} of each rank's quarter (= key tiles
# {4r, 4r+1}), chunk B blocks {2,3}.  Attention processes all A tiles
# first so it can start as soon as chunk A lands.
KC_ORDER = [4 * b + m for b in range(4) for m in range(2)] + \
           [4 * b + m for b in range(4) for m in range(2, 4)]

_NC_CACHE: dict = {}

# set by callers (e.g. test.py) to capture a profile; results of the last run
TRACE = False
TRACE_CORES = None          # e.g. [0] or list(range(8))
LAST_RESULTS = None


def _n_list(mode: str) -> list[int]:
    """Moving-operand width (in q columns, suffix of the 512) per key tile."""
    if mode == "causal":
        return [32 * (16 - kc) for kc in range(NKC)]
    return [512] * NKC


def _build(mode: str):
    mask_mul = mode != "none"
    n_of = _n_list(mode)

    nc = bacc.Bacc("TRN2", target_bir_lowering=False, debug=False,
                   num_devices=NCORES)

    # ---- I/O (host-prepared layouts; all contiguous-DMA friendly) ----
    wq = nc.declare_dram_parameter("wq", [NIC, 128, D], BF16, isOutput=False)
    qt = nc.declare_dram_parameter("qt", [128, NIC * RPC], BF16, isOutput=False)
    # k/v: only this core's 512-token quarter (projected here, all-gathered)
    kt = nc.declare_dram_parameter("kt", [128, NIC * 512], BF16, isOutput=False)
    vt = nc.declare_dram_parameter("vt", [4, 128, NIC * 128], BF16, isOutput=False)
    wk = nc.declare_dram_parameter("wk", [HKV, 128, NIC * 128], BF16, isOutput=False)
    wv = nc.declare_dram_parameter("wv", [128, NIC * DKV], BF16, isOutput=False)
    wo = nc.declare_dram_parameter("wo", [4, 128, H * 512], BF16, isOutput=False)
    cosq = nc.declare_dram_parameter("cosq", [128, RPC], BF16, isOutput=False)
    sinq = nc.declare_dram_parameter("sinq", [128, RPC], BF16, isOutput=False)
    # cos/sin for this core's own k-token quarter
    cosk = nc.declare_dram_parameter("cosk", [128, 512], BF16, isOutput=False)
    sink = nc.declare_dram_parameter("sink", [128, 512], BF16, isOutput=False)
    pswap = nc.declare_dram_parameter("pswap", [128, 128], BF16, isOutput=False)
    if mode == "causal":
        mdiag = nc.declare_dram_parameter("mdiag", [128, 32], BF16,
                                          isOutput=False)
    if mode == "mask":
        m01 = nc.declare_dram_parameter("m01", [128, NKC * RPC], BF16,
                                        isOutput=False)
    out = nc.declare_dram_parameter("out", [RPC, D], F32, isOutput=True)

    with tile.TileContext(nc) as tc:
        with (
            tc.tile_pool(name="res", bufs=1) as res,          # resident
            tc.tile_pool(name="stream2m", bufs=2) as stream2m,  # 2MB blocks
            tc.tile_pool(name="stream05", bufs=3) as stream05,  # 0.5MB blocks
            tc.tile_pool(name="small", bufs=3) as small,
            tc.tile_pool(name="probs", bufs=8) as probsp,
            tc.tile_pool(name="bcast", bufs=2) as bcastp,
            tc.tile_pool(name="dram", bufs=1, space="DRAM") as dramp,
            tc.tile_pool(name="psmm", bufs=4, space="PSUM") as psmm,
            tc.tile_pool(name="psacc", bufs=2, space="PSUM") as psacc,
            tc.tile_pool(name="pssum", bufs=2, space="PSUM") as pssum,
        ):
            # ---------------- resident tiles (DMAs staged per phase) -------
            # K-path first so the first matmul isn't stuck behind bulk loads
            coskq_t = res.tile([128, 512], BF16)
            nc.sync.dma_start(out=coskq_t, in_=cosk[:, :])
            sinkq_t = res.tile([128, 512], BF16)
            nc.sync.dma_start(out=sinkq_t, in_=sink[:, :])
            pswap_t = res.tile([128, 128], BF16)
            nc.sync.dma_start(out=pswap_t, in_=pswap[:, :])
            ones_t = res.tile([128, 1], BF16)
            nc.vector.memset(ones_t, 1.0)
            if mode == "causal":
                mdiag_t = res.tile([128, 32], BF16)
                nc.sync.dma_start(out=mdiag_t, in_=mdiag[:, :])
            # allocated here (tag order: qts before outu_a), loaded later
            qts = res.tile([128, NIC, RPC], BF16)

            qhs = res.tile([128, H, RPC], BF16)     # rope'd q, [dh, h, rows]
            khs = res.tile([128, HKV, S], BF16)     # rope'd k, [dh, hk, keys]
            vhs = res.tile([128, 16, DKV], BF16)    # v heads, [tok%128, tokc, kv]
            # outu_a shares qts's slot: qts is dead once phase A finishes.
            # split 12/4 so phase D's early matmuls (h<12) don't dep-chain
            # behind the last normalization batch (h>=12).
            outu_a = res.tile([128, 12, RPC], BF16, tag="qts")
            outu_b = res.tile([128, 4, RPC], BF16)

            def outu(h):
                return outu_a[:, h, :] if h < 12 else outu_b[:, h - 12, :]
            # normalization batches: heads [0:8], [8:12], [12:16]
            NB = [(0, 8), (8, 12), (12, 16)]
            sums_g = [res.tile([8, RPC], F32, name=f"sums{g}", tag=f"sums{g}")
                      for g in range(len(NB))]
            rec_g = [res.tile([8, RPC], F32, name=f"rec{g}", tag=f"rec{g}")
                     for g in range(len(NB))]
            sums_dram = dramp.tile([16, RPC], F32)
            rec_dram = dramp.tile([16, RPC], F32)
            khs_own = res.tile([128, HKV, 512], BF16)
            vhs_own = res.tile([128, 4, DKV], BF16)
            # 2-chunk staging: chunk m = own 128-token blocks {2m, 2m+1}
            # [128, 0:1024] = K (4 hk x 256 tok), [128, 1024:2048] = V
            kv_cin = dramp.tile([2, 128, 2048], BF16)
            # [chunk, rank, 128, 2048] so each chunk's gather output is
            # contiguous
            kv_cout = dramp.tile([2, 4, 128, 2048], BF16)

            def rope(dst, x_bf, ps_pool, cos_ap, sin_ap, n):
                """dst = x*cos + pairswap(x)*sin  (signs baked into sin)."""
                y_ps = ps_pool.tile([128, 512], F32, tag="mm")
                # moving operand max 1024 bf16 per matmul
                assert n <= 512
                nc.tensor.matmul(y_ps[:, :n], pswap_t, x_bf, start=True,
                                 stop=True)
                t1 = small.tile([128, 512], BF16, tag="t1")
                nc.vector.tensor_mul(t1[:, :n], x_bf, cos_ap)
                t2 = small.tile([128, 512], BF16, tag="t2")
                nc.vector.tensor_mul(t2[:, :n], y_ps[:, :n], sin_ap)
                nc.vector.tensor_add(dst, t1[:, :n], t2[:, :n])

            # ------- Phase B: K/V proj for OWN 512-token quarter + RoPE -----
            # (first, so the chunked all-gather overlaps Q proj + attention)
            kmov = stream2m.tile([128, NIC, 512], BF16, tag="s2m")
            # split the load so the first matmul starts ASAP
            for icq in range(4):
                nc.sync.dma_start(
                    out=kmov[:, 4 * icq:4 * icq + 4, :],
                    in_=kt[:, 4 * icq * 512:(4 * icq + 4) * 512].rearrange(
                        "p (i m) -> p i m", i=4))
            for hk in range(HKV):
                wk_all = stream05.tile([128, NIC, 128], BF16, tag="s05")
                nc.sync.dma_start(out=wk_all, in_=wk[hk].rearrange(
                    "p (i m) -> p i m", i=NIC))
                ps = psmm.tile([128, 512], F32, tag="mm")
                for ic in range(NIC):
                    nc.tensor.matmul(ps, wk_all[:, ic, :],
                                     kmov[:, ic, :],
                                     start=(ic == 0), stop=(ic == NIC - 1))
                xk = small.tile([128, 512], BF16, tag="xq")
                nc.scalar.copy(xk, ps)
                rope(khs_own[:, hk, :], xk, psmm, coskq_t, sinkq_t, 512)

            wvs = res.tile([128, NIC, DKV], BF16)
            nc.sync.dma_start(out=wvs, in_=wv[:, :].rearrange(
                "p (i n) -> p i n", i=NIC))
            for j in range(4):            # own 128-token blocks (V stationary)
                vmov = stream05.tile([128, NIC, 128], BF16, tag="s05")
                nc.sync.dma_start(out=vmov, in_=vt[j].rearrange(
                    "p (i m) -> p i m", i=NIC))
                ps = psmm.tile([128, 512], F32, tag="mm")
                for ic in range(NIC):
                    nc.tensor.matmul(ps, vmov[:, ic, :],
                                     wvs[:, ic, :],
                                     start=(ic == 0), stop=(ic == NIC - 1))
                nc.vector.tensor_copy(vhs_own[:, j, :], ps)
                if j % 2 == 0:
                    continue
                # stage + all-gather chunk m = blocks {2m, 2m+1}
                m = j // 2
                nc.sync.dma_start(
                    out=kv_cin[m, :, 0:1024].rearrange("p (h m) -> p h m",
                                                       h=HKV),
                    in_=khs_own[:, :, 256 * m:256 * (m + 1)])
                nc.sync.dma_start(
                    out=kv_cin[m, :, 1024:2048].rearrange("p (v m) -> p v m",
                                                          v=2),
                    in_=vhs_own[:, 2 * m:2 * m + 2, :])
                nc.gpsimd.collective_compute(
                    "AllGather", mybir.AluOpType.bypass,
                    replica_groups=[[0, 1, 2, 3], [4, 5, 6, 7]],
                    ins=[kv_cin[m]], outs=[kv_cout[m]])
                # unstage: chunk m of rank r covers key tiles {4r+2m, 4r+2m+1}
                for r in range(4):
                    nc.sync.dma_start(
                        out=khs[:, :, 512 * r + 256 * m:512 * r + 256 * (m + 1)],
                        in_=kv_cout[m, r, :, 0:1024].rearrange(
                            "p (h x) -> p h x", h=HKV))
                    nc.sync.dma_start(
                        out=vhs[:, 4 * r + 2 * m:4 * r + 2 * m + 2, :],
                        in_=kv_cout[m, r, :, 1024:2048].rearrange(
                            "p (v x) -> p v x", v=2))

            # ---------------- Phase A: Q-proj + RoPE ----------------
            nc.sync.dma_start(out=qts, in_=qt[:, :].rearrange(
                "p (i m) -> p i m", i=NIC))
            cosq_t = res.tile([128, RPC], BF16)
            nc.sync.dma_start(out=cosq_t, in_=cosq[:, :])
            sinq_t = res.tile([128, RPC], BF16)
            nc.sync.dma_start(out=sinq_t, in_=sinq[:, :])
            for oc in range(H):
                wq_all = stream05.tile([128, NIC, 128], BF16, tag="s05")
                nc.sync.dma_start(out=wq_all, in_=wq[oc].rearrange(
                    "p (i m) -> p i m", i=NIC))
                ps = psmm.tile([128, 512], F32, tag="mm")
                for ic in range(NIC):
                    nc.tensor.matmul(ps, wq_all[:, ic, :],
                                     qts[:, ic, :],
                                     start=(ic == 0), stop=(ic == NIC - 1))
                xq = small.tile([128, 512], BF16, tag="xq")
                nc.scalar.copy(xq, ps)
                rope(qhs[:, oc, :], xq, psmm, cosq_t, sinq_t, RPC)

            # ---------------- Phase C: attention per head ----------------
            if mode == "mask":
                m01s = res.tile([128, NKC, RPC], BF16)
                nc.sync.dma_start(out=m01s, in_=m01[:, :].rearrange(
                    "p (k m) -> p k m", k=NKC))

            def normalize_batch(g):
                """reciprocal + broadcast + in-place normalize for the heads
                of batch g (their sums are already in sums_dram)."""
                a, bnd = NB[g]
                m = bnd - a
                nc.sync.dma_start(out=sums_g[g][:m, :],
                                  in_=sums_dram[a:bnd, :])
                nc.vector.reciprocal(rec_g[g][:m, :], sums_g[g][:m, :])
                nc.sync.dma_start(out=rec_dram[a:bnd, :], in_=rec_g[g][:m, :])
                for h in range(a, bnd):
                    recb = bcastp.tile([128, RPC], F32, tag="bc")
                    nc.sync.dma_start(
                        out=recb,
                        in_=rec_dram[h:h + 1, :].to_broadcast([128, RPC]))
                    nc.vector.tensor_mul(outu(h), outu(h), recb)

            LA = 1                # kc-step lookahead (software pipeline)

            # heads processed in pairs sharing a kv head: the K/V stationary
            # is identical for both, and for n <= 256 both heads' scores
            # pack into one PSUM bank so one ACT covers both exps.
            for hp in range(H // 2):
                h0, h1 = 2 * hp, 2 * hp + 1
                hk = h0 // G
                ps_o0 = psacc.tile([128, 512], F32, tag="acc", name="ps_o0")
                ps_o1 = psacc.tile([128, 512], F32, tag="acc", name="ps_o1")
                ps_s0 = pssum.tile([1, 512], F32, tag="sum", name="ps_s0")
                ps_s1 = pssum.tile([1, 512], F32, tag="sum", name="ps_s1")
                pending = {}

                def issue_scores(i, h0=h0, h1=h1, hk=hk, pending=pending):
                    kc = KC_ORDER[i]
                    n = n_of[kc]
                    lo = RPC - n          # suffix columns
                    kap = khs[:, hk, kc * 128:(kc + 1) * 128]
                    packed = 2 * n <= 512
                    if packed:
                        ps_sc = psmm.tile([128, 512], F32, tag="mm")
                        nc.tensor.matmul(ps_sc[:, :n], kap, qhs[:, h0, lo:],
                                         start=True, stop=False,
                                         skip_group_check=True)
                        nc.tensor.matmul(ps_sc[:, n:2 * n], kap,
                                         qhs[:, h1, lo:],
                                         start=False, stop=True,
                                         skip_group_check=True)
                        probs = probsp.tile([128, 512], BF16, tag="pr")
                        nc.scalar.activation(
                            probs[:, :2 * n], ps_sc[:, :2 * n],
                            mybir.ActivationFunctionType.Exp, scale=SCALE)
                        pr0, pr1 = probs[:, :n], probs[:, n:2 * n]
                        d0, d1 = probs[:, 0:32], probs[:, n:n + 32]
                    else:
                        ps_a = psmm.tile([128, 512], F32, tag="mm")
                        nc.tensor.matmul(ps_a[:, :n], kap, qhs[:, h0, lo:],
                                         start=True, stop=True,
                                         skip_group_check=True)
                        ps_b = psmm.tile([128, 512], F32, tag="mm")
                        nc.tensor.matmul(ps_b[:, :n], kap, qhs[:, h1, lo:],
                                         start=True, stop=True,
                                         skip_group_check=True)
                        probs0 = probsp.tile([128, 512], BF16, tag="pr")
                        nc.scalar.activation(
                            probs0[:, :n], ps_a[:, :n],
                            mybir.ActivationFunctionType.Exp, scale=SCALE)
                        probs1 = probsp.tile([128, 512], BF16, tag="pr")
                        nc.scalar.activation(
                            probs1[:, :n], ps_b[:, :n],
                            mybir.ActivationFunctionType.Exp, scale=SCALE)
                        pr0, pr1 = probs0[:, :n], probs1[:, :n]
                        d0, d1 = probs0[:, 0:32], probs1[:, 0:32]
                    if mode == "causal":
                        # only the first 32 suffix columns (the diagonal
                        # 32-row q-block) are partially masked
                        nc.vector.tensor_mul(d0, d0, mdiag_t)
                        nc.vector.tensor_mul(d1, d1, mdiag_t)
                    elif mask_mul:
                        map_ = m01s[:, kc, lo:]
                        nc.vector.tensor_mul(pr0, pr0, map_)
                        nc.vector.tensor_mul(pr1, pr1, map_)
                    pending[i] = (pr0, pr1, kc, n, lo)

                for i in range(LA):
                    issue_scores(i)
                for idx in range(NKC):
                    if idx + LA < NKC:
                        issue_scores(idx + LA)
                    pr0, pr1, kc, n, lo = pending.pop(idx)
                    first = idx == 0
                    last = idx == NKC - 1
                    vap = vhs[:, kc, hk * 128:(hk + 1) * 128]
                    nc.tensor.matmul(ps_s0[:, lo:], ones_t, pr0,
                                     start=first, stop=last,
                                     skip_group_check=True)
                    nc.tensor.matmul(ps_s1[:, lo:], ones_t, pr1,
                                     start=first, stop=last,
                                     skip_group_check=True)
                    nc.tensor.matmul(ps_o0[:, lo:], vap, pr0,
                                     start=first, stop=last,
                                     skip_group_check=True)
                    nc.tensor.matmul(ps_o1[:, lo:], vap, pr1,
                                     start=first, stop=last,
                                     skip_group_check=True)
                for h, ps_s, ps_o in ((h0, ps_s0, ps_o0), (h1, ps_s1, ps_o1)):
                    sm1 = small.tile([1, RPC], F32, tag="sm1", bufs=2)
                    nc.vector.tensor_copy(sm1, ps_s)
                    nc.sync.dma_start(out=sums_dram[h:h + 1, :], in_=sm1)
                    nc.vector.tensor_copy(outu(h), ps_o)
                if h1 == 7:
                    normalize_batch(0)
                elif h1 == 11:
                    normalize_batch(1)
            normalize_batch(2)

            # ---------------- Phase D: out-projection ----------------
            for oc in range(4):
                wo_all = stream2m.tile([128, H, 512], BF16, tag="s2m")
                nc.sync.dma_start(out=wo_all, in_=wo[oc].rearrange(
                    "p (h m) -> p h m", h=H))
                for qc in range(4):
                    ps_f = psmm.tile([128, 512], F32, tag="mm")
                    for h in range(H):
                        lh = outu_a[:, h, qc * 128:(qc + 1) * 128] if h < 12 \
                            else outu_b[:, h - 12, qc * 128:(qc + 1) * 128]
                        nc.tensor.matmul(
                            ps_f, lh, wo_all[:, h, :],
                            start=(h == 0), stop=(h == H - 1))
                    fin = small.tile([128, 512], F32, tag="fin")
                    nc.vector.tensor_copy(fin, ps_f)
                    nc.sync.dma_start(
                        out=out[qc * 128:(qc + 1) * 128,
                                oc * 512:(oc + 1) * 512],
                        in_=fin)

    nc.compile()
    return nc


def _get_nc(mode: str):
    if mode not in _NC_CACHE:
        _NC_CACHE[mode] = _build(mode)
    return _NC_CACHE[mode]


def _core_rows(mode: str, r: int) -> np.ndarray:
    """Global (within-batch) q-row indices owned by quarter r, ascending.

    causal: 16 interleaved 32-row blocks {4j + r : j} -> exact suffix
    causality, identical shapes on every core.  other modes: contiguous.
    """
    if mode == "causal":
        return np.concatenate([np.arange(32 * (4 * j + r), 32 * (4 * j + r + 1))
                               for j in range(16)])
    return np.arange(r * RPC, (r + 1) * RPC)


def kernel(q, k, v, mask, freqs, W_q, W_k, W_v, W_o):
    q = np.asarray(q, dtype=np.float32)
    k = np.asarray(k, dtype=np.float32)
    v = np.asarray(v, dtype=np.float32)
    mask = np.asarray(mask, dtype=np.float32)
    freqs = np.asarray(freqs, dtype=np.float32)
    W_q = np.asarray(W_q, dtype=np.float32)
    W_k = np.asarray(W_k, dtype=np.float32)
    W_v = np.asarray(W_v, dtype=np.float32)
    W_o = np.asarray(W_o, dtype=np.float32)

    # ---- mask mode detection ----
    nz = mask != 0
    if nz.all():
        mode = "none"
    else:
        tril = np.tril(np.ones((S, S), dtype=bool))
        mode = "causal" if all(np.array_equal(nz[b], tril) for b in range(B)) \
            else "mask"

    # ---- shared host precomputation ----
    c_full = np.cos(freqs)                      # [S, 64]
    s_full = np.sin(freqs)
    sgn = np.tile(np.array([-1.0, 1.0], np.float32), DH // 2)  # [-,+,-,+...]
    cosk_h = np.repeat(c_full, 2, axis=1).T.astype(BF)          # [128, S]
    sink_h = (np.repeat(s_full, 2, axis=1) * sgn).T.astype(BF)

    psw = np.zeros((128, 128), np.float32)
    idx = np.arange(128)
    psw[idx, idx ^ 1] = 1.0
    psw = psw.astype(BF)

    # weight layouts
    # wq[oc, p, i*128+m] = W_q[oc*128+m, i*128+p]
    wq_h = np.ascontiguousarray(
        W_q.reshape(H, 128, NIC, 128).transpose(0, 3, 2, 1)
        .reshape(H, 128, D)).astype(BF)
    # wk[hk, p, i*128+m] = W_k[hk*128+m, i*128+p]
    wk_h = np.ascontiguousarray(
        W_k.reshape(HKV, 128, NIC, 128).transpose(0, 3, 2, 1)
        .reshape(HKV, 128, D)).astype(BF)
    # wv[p, i*512+n] = W_v[n, i*128+p]
    wv_h = np.ascontiguousarray(
        W_v.reshape(DKV, NIC, 128).transpose(2, 1, 0).reshape(128, NIC * DKV)
    ).astype(BF)
    # wo[oc, p, h*512+m] = W_o[oc*512+m, h*128+p]
    wo_h = np.ascontiguousarray(
        W_o.reshape(4, 512, H, 128).transpose(0, 3, 2, 1).reshape(4, 128, -1)
    ).astype(BF)

    # k/v: each core only gets its own 512-token quarter (gathered on device)
    # kt[p, i*512+t] = k[b, tq*512+t, i*128+p] for quarter tq
    kt_b = []   # [B][4] quarters
    vt_b = []
    for b in range(B):
        kt_b.append([np.ascontiguousarray(
            k[b, tq * 512:(tq + 1) * 512].reshape(512, NIC, 128)
            .transpose(2, 1, 0).reshape(128, NIC * 512)).astype(BF)
            for tq in range(4)])
        # vt[j, p, i*128+t] = v[b, tq*512 + j*128+t, i*128+p]
        vt_b.append([np.ascontiguousarray(
            v[b, tq * 512:(tq + 1) * 512].reshape(4, 128, NIC, 128)
            .transpose(0, 3, 2, 1).reshape(4, 128, NIC * 128)).astype(BF)
            for tq in range(4)])

    in_maps = []
    rows_all = []
    for c in range(NCORES):
        b, r = divmod(c, 4)
        rows = _core_rows(mode, r)
        rows_all.append((b, rows))
        # qt[p, i*512+t] = q[b, rows[t], i*128+p]
        qsl = q[b][rows]                       # [512, D]
        qt_h = np.ascontiguousarray(
            qsl.reshape(RPC, NIC, 128).transpose(2, 1, 0).reshape(128, -1)
        ).astype(BF)
        cq = np.repeat(c_full[rows], 2, axis=1).T.astype(BF)      # [128, 512]
        sq = (np.repeat(s_full[rows], 2, axis=1) * sgn).T.astype(BF)
        im = {
            "wq": wq_h, "qt": qt_h, "kt": kt_b[b][r], "vt": vt_b[b][r],
            "wk": wk_h, "wv": wv_h, "wo": wo_h,
            "cosq": cq, "sinq": sq,
            "cosk": np.ascontiguousarray(cosk_h[:, r * 512:(r + 1) * 512]),
            "sink": np.ascontiguousarray(sink_h[:, r * 512:(r + 1) * 512]),
            "pswap": psw,
        }
        if mode == "causal":
            # diagonal 32-col block mask: keep key p of tile kc for local
            # row i of block j=kc  <=>  p <= 32*r + i  (same for all kc)
            pp = np.arange(128)[:, None]
            ii = np.arange(32)[None, :]
            im["mdiag"] = (pp <= 32 * r + ii).astype(BF)
        elif mode == "mask":
            # m01[p, kc*512+m] = (mask[b, rows[m], kc*128+p] != 0)
            msl = nz[b][rows]                  # [512, S] bool
            m01_h = np.ascontiguousarray(
                msl.T.reshape(NKC, 128, RPC).transpose(1, 0, 2)
                .reshape(128, -1)).astype(BF)
            im["m01"] = m01_h
        in_maps.append(im)

    nc = _get_nc(mode)
    kwargs = {}
    if TRACE:
        kwargs["trace"] = True
        if TRACE_CORES:
            kwargs["trace_cores"] = list(TRACE_CORES)
    results = run_bass_kernel_spmd(nc, in_maps, core_ids=list(range(NCORES)),
                                   **kwargs)
    global LAST_RESULTS
    LAST_RESULTS = results

    full = np.empty((B, S, D), np.float32)
    for c in range(NCORES):
        b, rows = rows_all[c]
        full[b, rows] = results.results[c]["out"]
    return full


# revision 19
# speedup vs baseline: 1.1257x; 1.0396x over previous
"""Trainium2 Bass kernel for MultiHead GQA attention (B=2, S=2048, D=2048,
H=16 query heads, HKV=4 kv heads, DH=128, RoPE, mask, out-proj).

Sharding: token-parallel across 8 cores. Core c handles batch c//4 and 512
query rows of it. Each core projects K/V for its own 512-token quarter
(all 4 kv heads), the quarters are all-gathered in 4 pipelined 128-token
chunks, and the core runs attention + out-proj for its rows. Host
reassembles. All matmuls bf16 with fp32 PSUM accumulation.

Causal handling (exact, SPMD-uniform): core r of its batch owns the 16
interleaved 32-row q-blocks {4j + r : j=0..15} (ascending). For key tile
kc the q-blocks that attend to it are exactly the suffix of blocks with
position j >= kc, i.e. a contiguous column suffix of width n = 32*(16-kc)
-- identical on every core. Only the first 32 columns of each suffix (the
diagonal block) are partially masked; they get multiplied by a per-core
[128, 32] 0/1 tile. This computes 34 128x128-tile-equivalents per head
(the exact causal minimum for a 4-way row split) vs 40 for the previous
128-row-block scheme.

Attention is computed transposed: scoresT[keys, q] = khT.T @ qhT per
128-key tile, exp on ScalarE (scale folded in), probs bf16, then
outT[dh, q] += v_tile.T @ probsT, and row-sums via a ones-stationary
matmul. outT feeds the out-projection directly as stationary operand.

The K/V all-gather is split into 4 collectives, one per 128-token block
of each rank's quarter: chunk m delivers key tiles {4r + m : r=0..3}.
Attention iterates kc in the order [0,4,8,12, 1,5,9,13, ...] so the
first kc group only needs chunk 0 -- the remaining chunks stream in
behind attention/Q-proj compute instead of serializing in front of it.

Mask modes (host-detected, compile-time): none / causal / mask as before;
"mask" computes the full rectangle (n=512) and multiplies by the 0/1 mask.
"""

import math

import numpy as np
import ml_dtypes

import concourse.bass as bass
import concourse.mybir as mybir
import concourse.tile as tile
from concourse import bacc
from concourse.bass_utils import run_bass_kernel_spmd

F32 = mybir.dt.float32
BF16 = mybir.dt.bfloat16
BF = ml_dtypes.bfloat16

B, S, D = 2, 2048, 2048
H, G = 16, 4
HKV = H // G            # 4
DH = D // H             # 128
DKV = D // G            # 512 (kv projection width)
NCORES = 8
RPC = S // 4            # 512 rows per core
NIC = D // 128          # 16 contraction chunks
NKC = S // 128          # 16 key tiles
SCALE = 1.0 / math.sqrt(DH)
# attention kc order: the K/V all-gather is split in 2 chunks; chunk A
# carries 128-token blocks {0,1} of each rank's quarter (= key tiles
# {4r, 4r+1}), chunk B blocks {2,3}.  Attention processes all A tiles
# first so it can start as soon as chunk A lands.
KC_ORDER = [4 * b + m for b in range(4) for m in range(2)] + \
           [4 * b + m for b in range(4) for m in range(2, 4)]

_NC_CACHE: dict = {}

# set by callers (e.g. test.py) to capture a profile; results of the last run
TRACE = False
TRACE_CORES = None          # e.g. [0] or list(range(8))
LAST_RESULTS = None


def _n_list(mode: str) -> list[int]:
    """Moving-operand width (in q columns, suffix of the 512) per key tile."""
    if mode == "causal":
        return [32 * (16 - kc) for kc in range(NKC)]
    return [512] * NKC


def _build(mode: str):
    mask_mul = mode != "none"
    n_of = _n_list(mode)

    nc = bacc.Bacc("TRN2", target_bir_lowering=False, debug=False,
                   num_devices=NCORES)

    # ---- I/O (host-prepared layouts; all contiguous-DMA friendly) ----
    wq = nc.declare_dram_parameter("wq", [NIC, 128, D], BF16, isOutput=False)
    qt = nc.declare_dram_parameter("qt", [128, NIC * RPC], BF16, isOutput=False)
    # k/v: only this core's 512-token quarter (projected here, all-gathered)
    kt = nc.declare_dram_parameter("kt", [128, NIC * 512], BF16, isOutput=False)
    vt = nc.declare_dram_parameter("vt", [4, 128, NIC * 128], BF16, isOutput=False)
    wk = nc.declare_dram_parameter("wk", [HKV, 128, NIC * 128], BF16, isOutput=False)
    wv = nc.declare_dram_parameter("wv", [128, NIC * DKV], BF16, isOutput=False)
    wo = nc.declare_dram_parameter("wo", [4, 128, H * 512], BF16, isOutput=False)
    cosq = nc.declare_dram_parameter("cosq", [128, RPC], BF16, isOutput=False)
    sinq = nc.declare_dram_parameter("sinq", [128, RPC], BF16, isOutput=False)
    # cos/sin for this core's own k-token quarter
    cosk = nc.declare_dram_parameter("cosk", [128, 512], BF16, isOutput=False)
    sink = nc.declare_dram_parameter("sink", [128, 512], BF16, isOutput=False)
    pswap = nc.declare_dram_parameter("pswap", [128, 128], BF16, isOutput=False)
    if mode == "causal":
        mdiag = nc.declare_dram_parameter("mdiag", [128, 32], BF16,
                                          isOutput=False)
    if mode == "mask":
        m01 = nc.declare_dram_parameter("m01", [128, NKC * RPC], BF16,
                                        isOutput=False)
    out = nc.declare_dram_parameter("out", [RPC, D], F32, isOutput=True)

    with tile.TileContext(nc) as tc:
        with (
            tc.tile_pool(name="res", bufs=1) as res,          # resident
            tc.tile_pool(name="stream2m", bufs=2) as stream2m,  # 2MB blocks
            tc.tile_pool(name="stream05", bufs=3) as stream05,  # 0.5MB blocks
            tc.tile_pool(name="small", bufs=3) as small,
            tc.tile_pool(name="probs", bufs=8) as probsp,
            tc.tile_pool(name="bcast", bufs=2) as bcastp,
            tc.tile_pool(name="dram", bufs=1, space="DRAM") as dramp,
            tc.tile_pool(name="psmm", bufs=4, space="PSUM") as psmm,
            tc.tile_pool(name="psacc", bufs=2, space="PSUM") as psacc,
            tc.tile_pool(name="pssum", bufs=2, space="PSUM") as pssum,
        ):
            # ---------------- resident tiles (DMAs staged per phase) -------
            # K-proj operands stream first so the first matmul fires ASAP;
            # the small constant tiles follow them in the queue.
            kmov = stream2m.tile([128, NIC, 512], BF16, tag="s2m")
            nc.sync.dma_start(
                out=kmov[:, 0:4, :],
                in_=kt[:, 0:4 * 512].rearrange("p (i m) -> p i m", i=4))
            wk0 = stream05.tile([128, NIC, 128], BF16, tag="s05")
            nc.sync.dma_start(out=wk0, in_=wk[0].rearrange(
                "p (i m) -> p i m", i=NIC))
            for icq in range(1, 4):
                nc.sync.dma_start(
                    out=kmov[:, 4 * icq:4 * icq + 4, :],
                    in_=kt[:, 4 * icq * 512:(4 * icq + 4) * 512].rearrange(
                        "p (i m) -> p i m", i=4))
            coskq_t = res.tile([128, 512], BF16)
            nc.sync.dma_start(out=coskq_t, in_=cosk[:, :])
            sinkq_t = res.tile([128, 512], BF16)
            nc.sync.dma_start(out=sinkq_t, in_=sink[:, :])
            pswap_t = res.tile([128, 128], BF16)
            nc.sync.dma_start(out=pswap_t, in_=pswap[:, :])
            ones_t = res.tile([128, 1], BF16)
            nc.vector.memset(ones_t, 1.0)
            if mode == "causal":
                mdiag_t = res.tile([128, 32], BF16)
                nc.sync.dma_start(out=mdiag_t, in_=mdiag[:, :])
            # allocated here (tag order: qts before outu_a), loaded later
            qts = res.tile([128, NIC, RPC], BF16)

            qhs = res.tile([128, H, RPC], BF16)     # rope'd q, [dh, h, rows]
            khs = res.tile([128, HKV, S], BF16)     # rope'd k, [dh, hk, keys]
            vhs = res.tile([128, 16, DKV], BF16)    # v heads, [tok%128, tokc, kv]
            # outu_a shares qts's slot: qts is dead once phase A finishes.
            # split 12/4 so phase D's early matmuls (h<12) don't dep-chain
            # behind the last normalization batch (h>=12).
            outu_a = res.tile([128, 12, RPC], BF16, tag="qts")
            outu_b = res.tile([128, 4, RPC], BF16)

            def outu(h):
                return outu_a[:, h, :] if h < 12 else outu_b[:, h - 12, :]
            # normalization batches: heads [0:8], [8:12], [12:16]
            NB = [(0, 8), (8, 12), (12, 16)]
            sums_g = [res.tile([8, RPC], F32, name=f"sums{g}", tag=f"sums{g}")
                      for g in range(len(NB))]
            rec_g = [res.tile([8, RPC], F32, name=f"rec{g}", tag=f"rec{g}")
                     for g in range(len(NB))]
            sums_dram = dramp.tile([16, RPC], F32)
            rec_dram = dramp.tile([16, RPC], F32)
            khs_own = res.tile([128, HKV, 512], BF16)
            vhs_own = res.tile([128, 4, DKV], BF16)
            # 2-chunk staging: chunk m = own 128-token blocks {2m, 2m+1}
            # [128, 0:1024] = K (4 hk x 256 tok), [128, 1024:2048] = V
            kv_cin = dramp.tile([2, 128, 2048], BF16)
            # [chunk, rank, 128, 2048] so each chunk's gather output is
            # contiguous
            kv_cout = dramp.tile([2, 4, 128, 2048], BF16)

            def mm_dedup(out, lhsT, rhs, **kw):
                """Matmul that reuses the PE's already-loaded stationary
                operand (the immediately preceding matmul used the same
                lhsT), skipping the redundant LDWEIGHTS."""
                inst = nc.tensor.matmul(out, lhsT, rhs, **kw)
                try:
                    inst.ins.ldweights = False
                except Exception:
                    pass
                return inst

            def rope(dst, x_bf, ps_pool, cos_ap, sin_ap, n):
                """dst = x*cos + pairswap(x)*sin  (signs baked into sin)."""
                y_ps = ps_pool.tile([128, 512], F32, tag="mm")
                # moving operand max 1024 bf16 per matmul
                assert n <= 512
                nc.tensor.matmul(y_ps[:, :n], pswap_t, x_bf, start=True,
                                 stop=True)
                t1 = small.tile([128, 512], BF16, tag="t1")
                nc.vector.tensor_mul(t1[:, :n], x_bf, cos_ap)
                t2 = small.tile([128, 512], BF16, tag="t2")
                nc.vector.tensor_mul(t2[:, :n], y_ps[:, :n], sin_ap)
                nc.vector.tensor_add(dst, t1[:, :n], t2[:, :n])

            # ------- Phase B: K/V proj for OWN 512-token quarter + RoPE -----
            # (first, so the chunked all-gather overlaps Q proj + attention)
            for hk in range(HKV):
                if hk == 0:
                    wk_all = wk0
                else:
                    wk_all = stream05.tile([128, NIC, 128], BF16, tag="s05")
                    nc.sync.dma_start(out=wk_all, in_=wk[hk].rearrange(
                        "p (i m) -> p i m", i=NIC))
                ps = psmm.tile([128, 512], F32, tag="mm")
                for ic in range(NIC):
                    nc.tensor.matmul(ps, wk_all[:, ic, :],
                                     kmov[:, ic, :],
                                     start=(ic == 0), stop=(ic == NIC - 1))
                xk = small.tile([128, 512], BF16, tag="xq")
                nc.scalar.copy(xk, ps)
                rope(khs_own[:, hk, :], xk, psmm, coskq_t, sinkq_t, 512)

            wvs = res.tile([128, NIC, DKV], BF16)
            nc.sync.dma_start(out=wvs, in_=wv[:, :].rearrange(
                "p (i n) -> p i n", i=NIC))
            for j in range(4):            # own 128-token blocks (V stationary)
                vmov = stream05.tile([128, NIC, 128], BF16, tag="s05")
                nc.sync.dma_start(out=vmov, in_=vt[j].rearrange(
                    "p (i m) -> p i m", i=NIC))
                ps = psmm.tile([128, 512], F32, tag="mm")
                for ic in range(NIC):
                    nc.tensor.matmul(ps, vmov[:, ic, :],
                                     wvs[:, ic, :],
                                     start=(ic == 0), stop=(ic == NIC - 1))
                nc.vector.tensor_copy(vhs_own[:, j, :], ps)
                if j % 2 == 0:
                    continue
                # stage + all-gather chunk m = blocks {2m, 2m+1}
                m = j // 2
                nc.sync.dma_start(
                    out=kv_cin[m, :, 0:1024].rearrange("p (h m) -> p h m",
                                                       h=HKV),
                    in_=khs_own[:, :, 256 * m:256 * (m + 1)])
                nc.sync.dma_start(
                    out=kv_cin[m, :, 1024:2048].rearrange("p (v m) -> p v m",
                                                          v=2),
                    in_=vhs_own[:, 2 * m:2 * m + 2, :])
                nc.gpsimd.collective_compute(
                    "AllGather", mybir.AluOpType.bypass,
                    replica_groups=[[0, 1, 2, 3], [4, 5, 6, 7]],
                    ins=[kv_cin[m]], outs=[kv_cout[m]])
                # unstage: chunk m of rank r covers key tiles {4r+2m, 4r+2m+1}
                for r in range(4):
                    nc.sync.dma_start(
                        out=khs[:, :, 512 * r + 256 * m:512 * r + 256 * (m + 1)],
                        in_=kv_cout[m, r, :, 0:1024].rearrange(
                            "p (h x) -> p h x", h=HKV))
                    nc.sync.dma_start(
                        out=vhs[:, 4 * r + 2 * m:4 * r + 2 * m + 2, :],
                        in_=kv_cout[m, r, :, 1024:2048].rearrange(
                            "p (v x) -> p v x", v=2))

            # ---------------- Phase A: Q-proj + RoPE ----------------
            nc.sync.dma_start(out=qts, in_=qt[:, :].rearrange(
                "p (i m) -> p i m", i=NIC))
            cosq_t = res.tile([128, RPC], BF16)
            nc.sync.dma_start(out=cosq_t, in_=cosq[:, :])
            sinq_t = res.tile([128, RPC], BF16)
            nc.sync.dma_start(out=sinq_t, in_=sinq[:, :])
            for oc in range(H):
                wq_all = stream05.tile([128, NIC, 128], BF16, tag="s05")
                nc.sync.dma_start(out=wq_all, in_=wq[oc].rearrange(
                    "p (i m) -> p i m", i=NIC))
                ps = psmm.tile([128, 512], F32, tag="mm")
                for ic in range(NIC):
                    nc.tensor.matmul(ps, wq_all[:, ic, :],
                                     qts[:, ic, :],
                                     start=(ic == 0), stop=(ic == NIC - 1))
                xq = small.tile([128, 512], BF16, tag="xq")
                nc.scalar.copy(xq, ps)
                rope(qhs[:, oc, :], xq, psmm, cosq_t, sinq_t, RPC)

            # ---------------- Phase C: attention per head ----------------
            if mode == "mask":
                m01s = res.tile([128, NKC, RPC], BF16)
                nc.sync.dma_start(out=m01s, in_=m01[:, :].rearrange(
                    "p (k m) -> p k m", k=NKC))

            def normalize_batch(g):
                """reciprocal + broadcast + in-place normalize for the heads
                of batch g (their sums are already in sums_dram)."""
                a, bnd = NB[g]
                m = bnd - a
                nc.sync.dma_start(out=sums_g[g][:m, :],
                                  in_=sums_dram[a:bnd, :])
                nc.vector.reciprocal(rec_g[g][:m, :], sums_g[g][:m, :])
                nc.sync.dma_start(out=rec_dram[a:bnd, :], in_=rec_g[g][:m, :])
                for h in range(a, bnd):
                    recb = bcastp.tile([128, RPC], F32, tag="bc")
                    nc.sync.dma_start(
                        out=recb,
                        in_=rec_dram[h:h + 1, :].to_broadcast([128, RPC]))
                    nc.vector.tensor_mul(outu(h), outu(h), recb)

            LA = 1                # kc-step lookahead (software pipeline)

            # heads processed in pairs sharing a kv head: the K/V stationary
            # is identical for both, and for n <= 256 both heads' scores
            # pack into one PSUM bank so one ACT covers both exps.
            for hp in range(H // 2):
                h0, h1 = 2 * hp, 2 * hp + 1
                hk = h0 // G
                ps_o0 = psacc.tile([128, 512], F32, tag="acc", name="ps_o0")
                ps_o1 = psacc.tile([128, 512], F32, tag="acc", name="ps_o1")
                ps_s0 = pssum.tile([1, 512], F32, tag="sum", name="ps_s0")
                ps_s1 = pssum.tile([1, 512], F32, tag="sum", name="ps_s1")
                pending = {}

                def issue_scores(i, h0=h0, h1=h1, hk=hk, pending=pending):
                    kc = KC_ORDER[i]
                    n = n_of[kc]
                    lo = RPC - n          # suffix columns
                    kap = khs[:, hk, kc * 128:(kc + 1) * 128]
                    packed = 2 * n <= 512
                    if packed:
                        ps_sc = psmm.tile([128, 512], F32, tag="mm")
                        nc.tensor.matmul(ps_sc[:, :n], kap, qhs[:, h0, lo:],
                                         start=True, stop=False,
                                         skip_group_check=True)
                        mm_dedup(ps_sc[:, n:2 * n], kap,
                                 qhs[:, h1, lo:],
                                 start=False, stop=True,
                                 skip_group_check=True)
                        probs = probsp.tile([128, 512], BF16, tag="pr")
                        nc.scalar.activation(
                            probs[:, :2 * n], ps_sc[:, :2 * n],
                            mybir.ActivationFunctionType.Exp, scale=SCALE)
                        pr0, pr1 = probs[:, :n], probs[:, n:2 * n]
                        d0, d1 = probs[:, 0:32], probs[:, n:n + 32]
                    else:
                        ps_a = psmm.tile([128, 512], F32, tag="mm")
                        nc.tensor.matmul(ps_a[:, :n], kap, qhs[:, h0, lo:],
                                         start=True, stop=True,
                                         skip_group_check=True)
                        ps_b = psmm.tile([128, 512], F32, tag="mm")
                        mm_dedup(ps_b[:, :n], kap, qhs[:, h1, lo:],
                                 start=True, stop=True,
                                 skip_group_check=True)
                        probs0 = probsp.tile([128, 512], BF16, tag="pr")
                        nc.scalar.activation(
                            probs0[:, :n], ps_a[:, :n],
                            mybir.ActivationFunctionType.Exp, scale=SCALE)
                        probs1 = probsp.tile([128, 512], BF16, tag="pr")
                        nc.scalar.activation(
                            probs1[:, :n], ps_b[:, :n],
                            mybir.ActivationFunctionType.Exp, scale=SCALE)
                        pr0, pr1 = probs0[:, :n], probs1[:, :n]
                        d0, d1 = probs0[:, 0:32], probs1[:, 0:32]
                    if mode == "causal":
                        # only the first 32 suffix columns (the diagonal
                        # 32-row q-block) are partially masked
                        nc.vector.tensor_mul(d0, d0, mdiag_t)
                        nc.vector.tensor_mul(d1, d1, mdiag_t)
                    elif mask_mul:
                        map_ = m01s[:, kc, lo:]
                        nc.vector.tensor_mul(pr0, pr0, map_)
                        nc.vector.tensor_mul(pr1, pr1, map_)
                    pending[i] = (pr0, pr1, kc, n, lo)

                for i in range(LA):
                    issue_scores(i)
                for idx in range(NKC):
                    if idx + LA < NKC:
                        issue_scores(idx + LA)
                    pr0, pr1, kc, n, lo = pending.pop(idx)
                    first = idx == 0
                    last = idx == NKC - 1
                    vap = vhs[:, kc, hk * 128:(hk + 1) * 128]
                    nc.tensor.matmul(ps_s0[:, lo:], ones_t, pr0,
                                     start=first, stop=last,
                                     skip_group_check=True)
                    mm_dedup(ps_s1[:, lo:], ones_t, pr1,
                             start=first, stop=last,
                             skip_group_check=True)
                    nc.tensor.matmul(ps_o0[:, lo:], vap, pr0,
                                     start=first, stop=last,
                                     skip_group_check=True)
                    mm_dedup(ps_o1[:, lo:], vap, pr1,
                             start=first, stop=last,
                             skip_group_check=True)
                for h, ps_s, ps_o in ((h0, ps_s0, ps_o0), (h1, ps_s1, ps_o1)):
                    sm1 = small.tile([1, RPC], F32, tag="sm1", bufs=2)
                    nc.vector.tensor_copy(sm1, ps_s)
                    nc.sync.dma_start(out=sums_dram[h:h + 1, :], in_=sm1)
                    nc.vector.tensor_copy(outu(h), ps_o)
                if h1 == 7:
                    normalize_batch(0)
                elif h1 == 11:
                    normalize_batch(1)
            normalize_batch(2)

            # ---------------- Phase D: out-projection ----------------
            for oc in range(4):
                wo_all = stream2m.tile([128, H, 512], BF16, tag="s2m")
                nc.sync.dma_start(out=wo_all, in_=wo[oc].rearrange(
                    "p (h m) -> p h m", h=H))
                for qc in range(4):
                    ps_f = psmm.tile([128, 512], F32, tag="mm")
                    for h in range(H):
                        lh = outu_a[:, h, qc * 128:(qc + 1) * 128] if h < 12 \
                            else outu_b[:, h - 12, qc * 128:(qc + 1) * 128]
                        nc.tensor.matmul(
                            ps_f, lh, wo_all[:, h, :],
                            start=(h == 0), stop=(h == H - 1))
                    fin = small.tile([128, 512], F32, tag="fin")
                    nc.vector.tensor_copy(fin, ps_f)
                    nc.sync.dma_start(
                        out=out[qc * 128:(qc + 1) * 128,
                                oc * 512:(oc + 1) * 512],
                        in_=fin)

    nc.compile()
    return nc


def _get_nc(mode: str):
    if mode not in _NC_CACHE:
        _NC_CACHE[mode] = _build(mode)
    return _NC_CACHE[mode]


def _core_rows(mode: str, r: int) -> np.ndarray:
    """Global (within-batch) q-row indices owned by quarter r, ascending.

    causal: 16 interleaved 32-row blocks {4j + r : j} -> exact suffix
    causality, identical shapes on every core.  other modes: contiguous.
    """
    if mode == "causal":
        return np.concatenate([np.arange(32 * (4 * j + r), 32 * (4 * j + r + 1))
                               for j in range(16)])
    return np.arange(r * RPC, (r + 1) * RPC)


def kernel(q, k, v, mask, freqs, W_q, W_k, W_v, W_o):
    q = np.asarray(q, dtype=np.float32)
    k = np.asarray(k, dtype=np.float32)
    v = np.asarray(v, dtype=np.float32)
    mask = np.asarray(mask, dtype=np.float32)
    freqs = np.asarray(freqs, dtype=np.float32)
    W_q = np.asarray(W_q, dtype=np.float32)
    W_k = np.asarray(W_k, dtype=np.float32)
    W_v = np.asarray(W_v, dtype=np.float32)
    W_o = np.asarray(W_o, dtype=np.float32)

    # ---- mask mode detection ----
    nz = mask != 0
    if nz.all():
        mode = "none"
    else:
        tril = np.tril(np.ones((S, S), dtype=bool))
        mode = "causal" if all(np.array_equal(nz[b], tril) for b in range(B)) \
            else "mask"

    # ---- shared host precomputation ----
    c_full = np.cos(freqs)                      # [S, 64]
    s_full = np.sin(freqs)
    sgn = np.tile(np.array([-1.0, 1.0], np.float32), DH // 2)  # [-,+,-,+...]
    cosk_h = np.repeat(c_full, 2, axis=1).T.astype(BF)          # [128, S]
    sink_h = (np.repeat(s_full, 2, axis=1) * sgn).T.astype(BF)

    psw = np.zeros((128, 128), np.float32)
    idx = np.arange(128)
    psw[idx, idx ^ 1] = 1.0
    psw = psw.astype(BF)

    # weight layouts
    # wq[oc, p, i*128+m] = W_q[oc*128+m, i*128+p]
    wq_h = np.ascontiguousarray(
        W_q.reshape(H, 128, NIC, 128).transpose(0, 3, 2, 1)
        .reshape(H, 128, D)).astype(BF)
    # wk[hk, p, i*128+m] = W_k[hk*128+m, i*128+p]
    wk_h = np.ascontiguousarray(
        W_k.reshape(HKV, 128, NIC, 128).transpose(0, 3, 2, 1)
        .reshape(HKV, 128, D)).astype(BF)
    # wv[p, i*512+n] = W_v[n, i*128+p]
    wv_h = np.ascontiguousarray(
        W_v.reshape(DKV, NIC, 128).transpose(2, 1, 0).reshape(128, NIC * DKV)
    ).astype(BF)
    # wo[oc, p, h*512+m] = W_o[oc*512+m, h*128+p]
    wo_h = np.ascontiguousarray(
        W_o.reshape(4, 512, H, 128).transpose(0, 3, 2, 1).reshape(4, 128, -1)
    ).astype(BF)

    # k/v: each core only gets its own 512-token quarter (gathered on device)
    # kt[p, i*512+t] = k[b, tq*512+t, i*128+p] for quarter tq
    kt_b = []   # [B][4] quarters
    vt_b = []
    for b in range(B):
        kt_b.append([np.ascontiguousarray(
            k[b, tq * 512:(tq + 1) * 512].reshape(512, NIC, 128)
            .transpose(2, 1, 0).reshape(128, NIC * 512)).astype(BF)
            for tq in range(4)])
        # vt[j, p, i*128+t] = v[b, tq*512 + j*128+t, i*128+p]
        vt_b.append([np.ascontiguousarray(
            v[b, tq * 512:(tq + 1) * 512].reshape(4, 128, NIC, 128)
            .transpose(0, 3, 2, 1).reshape(4, 128, NIC * 128)).astype(BF)
            for tq in range(4)])

    in_maps = []
    rows_all = []
    for c in range(NCORES):
        b, r = divmod(c, 4)
        rows = _core_rows(mode, r)
        rows_all.append((b, rows))
        # qt[p, i*512+t] = q[b, rows[t], i*128+p]
        qsl = q[b][rows]                       # [512, D]
        qt_h = np.ascontiguousarray(
            qsl.reshape(RPC, NIC, 128).transpose(2, 1, 0).reshape(128, -1)
        ).astype(BF)
        cq = np.repeat(c_full[rows], 2, axis=1).T.astype(BF)      # [128, 512]
        sq = (np.repeat(s_full[rows], 2, axis=1) * sgn).T.astype(BF)
        im = {
            "wq": wq_h, "qt": qt_h, "kt": kt_b[b][r], "vt": vt_b[b][r],
            "wk": wk_h, "wv": wv_h, "wo": wo_h,
            "cosq": cq, "sinq": sq,
            "cosk": np.ascontiguousarray(cosk_h[:, r * 512:(r + 1) * 512]),
            "sink": np.ascontiguousarray(sink_h[:, r * 512:(r + 1) * 512]),
            "pswap": psw,
        }
        if mode == "causal":
            # diagonal 32-col block mask: keep key p of tile kc for local
            # row i of block j=kc  <=>  p <= 32*r + i  (same for all kc)
            pp = np.arange(128)[:, None]
            ii = np.arange(32)[None, :]
            im["mdiag"] = (pp <= 32 * r + ii).astype(BF)
        elif mode == "mask":
            # m01[p, kc*512+m] = (mask[b, rows[m], kc*128+p] != 0)
            msl = nz[b][rows]                  # [512, S] bool
            m01_h = np.ascontiguousarray(
                msl.T.reshape(NKC, 128, RPC).transpose(1, 0, 2)
                .reshape(128, -1)).astype(BF)
            im["m01"] = m01_h
        in_maps.append(im)

    nc = _get_nc(mode)
    kwargs = {}
    if TRACE:
        kwargs["trace"] = True
        if TRACE_CORES:
            kwargs["trace_cores"] = list(TRACE_CORES)
    results = run_bass_kernel_spmd(nc, in_maps, core_ids=list(range(NCORES)),
                                   **kwargs)
    global LAST_RESULTS
    LAST_RESULTS = results

    full = np.empty((B, S, D), np.float32)
    for c in range(NCORES):
        b, rows = rows_all[c]
        full[b, rows] = results.results[c]["out"]
    return full


# revision 25
# speedup vs baseline: 1.1403x; 1.0130x over previous
"""Trainium2 Bass kernel for MultiHead GQA attention (B=2, S=2048, D=2048,
H=16 query heads, HKV=4 kv heads, DH=128, RoPE, mask, out-proj).

Sharding: token-parallel across 8 cores. Core c handles batch c//4 and 512
query rows of it. Each core projects K/V for its own 512-token quarter
(all 4 kv heads), the quarters are all-gathered in 4 pipelined 128-token
chunks, and the core runs attention + out-proj for its rows. Host
reassembles. All matmuls bf16 with fp32 PSUM accumulation.

Causal handling (exact, SPMD-uniform): core r of its batch owns the 16
interleaved 32-row q-blocks {4j + r : j=0..15} (ascending). For key tile
kc the q-blocks that attend to it are exactly the suffix of blocks with
position j >= kc, i.e. a contiguous column suffix of width n = 32*(16-kc)
-- identical on every core. Only the first 32 columns of each suffix (the
diagonal block) are partially masked; they get multiplied by a per-core
[128, 32] 0/1 tile. This computes 34 128x128-tile-equivalents per head
(the exact causal minimum for a 4-way row split) vs 40 for the previous
128-row-block scheme.

Attention is computed transposed: scoresT[keys, q] = khT.T @ qhT per
128-key tile, exp on ScalarE (scale folded in), probs bf16, then
outT[dh, q] += v_tile.T @ probsT, and row-sums via a ones-stationary
matmul. outT feeds the out-projection directly as stationary operand.

The K/V all-gather is split into 4 collectives, one per 128-token block
of each rank's quarter: chunk m delivers key tiles {4r + m : r=0..3}.
Attention iterates kc in the order [0,4,8,12, 1,5,9,13, ...] so the
first kc group only needs chunk 0 -- the remaining chunks stream in
behind attention/Q-proj compute instead of serializing in front of it.

Mask modes (host-detected, compile-time): none / causal / mask as before;
"mask" computes the full rectangle (n=512) and multiplies by the 0/1 mask.
"""

import math

import numpy as np
import ml_dtypes

import concourse.bass as bass
import concourse.mybir as mybir
import concourse.tile as tile
from concourse import bacc
from concourse.bass_utils import run_bass_kernel_spmd

F32 = mybir.dt.float32
BF16 = mybir.dt.bfloat16
BF = ml_dtypes.bfloat16

B, S, D = 2, 2048, 2048
H, G = 16, 4
HKV = H // G            # 4
DH = D // H             # 128
DKV = D // G            # 512 (kv projection width)
NCORES = 8
RPC = S // 4            # 512 rows per core
NIC = D // 128          # 16 contraction chunks
NKC = S // 128          # 16 key tiles
SCALE = 1.0 / math.sqrt(DH)
# attention kc order: the K/V all-gather is split in 2 chunks; chunk A
# carries 128-token blocks {0,1,2} of each rank's quarter (= key tiles
# {4r, 4r+1, 4r+2}), chunk B block {3}.  Attention processes all A tiles
# first so it can start as soon as chunk A lands; the small B chunk
# arrives while the A tiles are being consumed.
CHUNK_BLKS = [(0, 3), (3, 4)]        # [lo, hi) own-token-block range per chunk
KC_ORDER = [4 * b + m for b in range(4) for m in range(3)] + \
           [4 * b + 3 for b in range(4)]

_NC_CACHE: dict = {}

# set by callers (e.g. test.py) to capture a profile; results of the last run
TRACE = False
TRACE_CORES = None          # e.g. [0] or list(range(8))
LAST_RESULTS = None


def _n_list(mode: str) -> list[int]:
    """Moving-operand width (in q columns, suffix of the 512) per key tile."""
    if mode == "causal":
        return [32 * (16 - kc) for kc in range(NKC)]
    return [512] * NKC


def _build(mode: str):
    mask_mul = mode != "none"
    n_of = _n_list(mode)

    nc = bacc.Bacc("TRN2", target_bir_lowering=False, debug=False,
                   num_devices=NCORES)

    # ---- I/O (host-prepared layouts; all contiguous-DMA friendly) ----
    wq = nc.declare_dram_parameter("wq", [NIC, 128, D], BF16, isOutput=False)
    qt = nc.declare_dram_parameter("qt", [128, NIC * RPC], BF16, isOutput=False)
    # k/v: only this core's 512-token quarter (projected here, all-gathered)
    kt = nc.declare_dram_parameter("kt", [128, NIC * 512], BF16, isOutput=False)
    vt = nc.declare_dram_parameter("vt", [4, 128, NIC * 128], BF16, isOutput=False)
    wk = nc.declare_dram_parameter("wk", [HKV, 128, NIC * 128], BF16, isOutput=False)
    wv = nc.declare_dram_parameter("wv", [128, NIC * DKV], BF16, isOutput=False)
    wo = nc.declare_dram_parameter("wo", [4, 128, H * 512], BF16, isOutput=False)
    cosq = nc.declare_dram_parameter("cosq", [128, RPC], BF16, isOutput=False)
    sinq = nc.declare_dram_parameter("sinq", [128, RPC], BF16, isOutput=False)
    # cos/sin for this core's own k-token quarter
    cosk = nc.declare_dram_parameter("cosk", [128, 512], BF16, isOutput=False)
    sink = nc.declare_dram_parameter("sink", [128, 512], BF16, isOutput=False)
    pswap = nc.declare_dram_parameter("pswap", [128, 128], BF16, isOutput=False)
    if mode == "causal":
        mdiag = nc.declare_dram_parameter("mdiag", [128, 32], BF16,
                                          isOutput=False)
    if mode == "mask":
        m01 = nc.declare_dram_parameter("m01", [128, NKC * RPC], BF16,
                                        isOutput=False)
    out = nc.declare_dram_parameter("out", [RPC, D], F32, isOutput=True)

    with tile.TileContext(nc) as tc:
        with (
            tc.tile_pool(name="res", bufs=1) as res,          # resident
            tc.tile_pool(name="stream2m", bufs=2) as stream2m,  # 2MB blocks
            tc.tile_pool(name="stream05", bufs=3) as stream05,  # 0.5MB blocks
            tc.tile_pool(name="small", bufs=3) as small,
            tc.tile_pool(name="probs", bufs=8) as probsp,
            tc.tile_pool(name="bcast", bufs=2) as bcastp,
            tc.tile_pool(name="dram", bufs=1, space="DRAM") as dramp,
            tc.tile_pool(name="psmm", bufs=5, space="PSUM") as psmm,
            tc.tile_pool(name="psacc", bufs=2, space="PSUM") as psacc,
            tc.tile_pool(name="pssum", bufs=1, space="PSUM") as pssum,
        ):
            # ---------------- resident tiles (DMAs staged per phase) -------
            # K-proj operands stream first so the first matmul fires ASAP;
            # the small constant tiles follow them in the queue.
            kmov = stream2m.tile([128, NIC, 512], BF16, tag="s2m")
            nc.sync.dma_start(
                out=kmov[:, 0:4, :],
                in_=kt[:, 0:4 * 512].rearrange("p (i m) -> p i m", i=4))
            wk0 = stream05.tile([128, NIC, 128], BF16, tag="s05")
            nc.sync.dma_start(out=wk0, in_=wk[0].rearrange(
                "p (i m) -> p i m", i=NIC))
            for icq in range(1, 4):
                nc.sync.dma_start(
                    out=kmov[:, 4 * icq:4 * icq + 4, :],
                    in_=kt[:, 4 * icq * 512:(4 * icq + 4) * 512].rearrange(
                        "p (i m) -> p i m", i=4))
            coskq_t = res.tile([128, 512], BF16)
            nc.sync.dma_start(out=coskq_t, in_=cosk[:, :])
            sinkq_t = res.tile([128, 512], BF16)
            nc.sync.dma_start(out=sinkq_t, in_=sink[:, :])
            pswap_t = res.tile([128, 128], BF16)
            nc.sync.dma_start(out=pswap_t, in_=pswap[:, :])
            ones_t = res.tile([128, 1], BF16)
            nc.vector.memset(ones_t, 1.0)
            if mode == "causal":
                mdiag_t = res.tile([128, 32], BF16)
                nc.sync.dma_start(out=mdiag_t, in_=mdiag[:, :])
            # allocated here (tag order: qts before outu_a), loaded later
            qts = res.tile([128, NIC, RPC], BF16)

            qhs = res.tile([128, H, RPC], BF16)     # rope'd q, [dh, h, rows]
            khs = res.tile([128, HKV, S], BF16)     # rope'd k, [dh, hk, keys]
            vhs = res.tile([128, 16, DKV], BF16)    # v heads, [tok%128, tokc, kv]
            # outu_a shares qts's slot: qts is dead once phase A finishes.
            # split 12/4 so phase D's early matmuls (h<12) don't dep-chain
            # behind the last normalization batch (h>=12).
            outu_a = res.tile([128, 12, RPC], BF16, tag="qts")
            outu_b = res.tile([128, 4, RPC], BF16)

            def outu(h):
                return outu_a[:, h, :] if h < 12 else outu_b[:, h - 12, :]
            # normalization batches: heads [0:8], [8:12], [12:16]
            NB = [(0, 8), (8, 12), (12, 16)]
            sums_g = [res.tile([8, RPC], F32, name=f"sums{g}", tag=f"sums{g}")
                      for g in range(len(NB))]
            rec_g = [res.tile([8, RPC], F32, name=f"rec{g}", tag=f"rec{g}")
                     for g in range(len(NB))]
            sums_dram = dramp.tile([16, RPC], F32)
            rec_dram = dramp.tile([16, RPC], F32)
            khs_own = res.tile([128, HKV, 512], BF16)
            vhs_own = res.tile([128, 4, DKV], BF16)
            # 2-chunk staging (3 blocks + 1 block); K first, then V
            kv_cinA = dramp.tile([128, 3072], BF16)
            kv_coutA = dramp.tile([4, 128, 3072], BF16)
            kv_cinB = dramp.tile([128, 1024], BF16)
            kv_coutB = dramp.tile([4, 128, 1024], BF16)

            def mm_dedup(out, lhsT, rhs, **kw):
                """Matmul that reuses the PE's already-loaded stationary
                operand (the immediately preceding matmul used the same
                lhsT), skipping the redundant LDWEIGHTS."""
                inst = nc.tensor.matmul(out, lhsT, rhs, **kw)
                try:
                    inst.ins.ldweights = False
                except Exception:
                    pass
                return inst

            def rope(dst, x_bf, ps_pool, cos_ap, sin_ap, n):
                """dst = x*cos + pairswap(x)*sin  (signs baked into sin)."""
                y_ps = ps_pool.tile([128, 512], F32, tag="mm")
                # moving operand max 1024 bf16 per matmul
                assert n <= 512
                nc.tensor.matmul(y_ps[:, :n], pswap_t, x_bf, start=True,
                                 stop=True)
                t1 = small.tile([128, 512], BF16, tag="t1")
                nc.vector.tensor_mul(t1[:, :n], x_bf, cos_ap)
                t2 = small.tile([128, 512], BF16, tag="t2")
                nc.vector.tensor_mul(t2[:, :n], y_ps[:, :n], sin_ap)
                nc.vector.tensor_add(dst, t1[:, :n], t2[:, :n])

            # ------- Phase B: K/V proj for OWN 512-token quarter + RoPE -----
            # (first, so the chunked all-gather overlaps Q proj + attention)
            for hk in range(HKV):
                if hk == 0:
                    wk_all = wk0
                else:
                    wk_all = stream05.tile([128, NIC, 128], BF16, tag="s05")
                    nc.sync.dma_start(out=wk_all, in_=wk[hk].rearrange(
                        "p (i m) -> p i m", i=NIC))
                ps = psmm.tile([128, 512], F32, tag="mm")
                for ic in range(NIC):
                    nc.tensor.matmul(ps, wk_all[:, ic, :],
                                     kmov[:, ic, :],
                                     start=(ic == 0), stop=(ic == NIC - 1))
                xk = small.tile([128, 512], BF16, tag="xq")
                nc.scalar.copy(xk, ps)
                rope(khs_own[:, hk, :], xk, psmm, coskq_t, sinkq_t, 512)

            wvs = res.tile([128, NIC, DKV], BF16)
            nc.sync.dma_start(out=wvs, in_=wv[:, :].rearrange(
                "p (i n) -> p i n", i=NIC))
            for j in range(4):            # own 128-token blocks (V stationary)
                vmov = stream05.tile([128, NIC, 128], BF16, tag="s05")
                nc.sync.dma_start(out=vmov, in_=vt[j].rearrange(
                    "p (i m) -> p i m", i=NIC))
                ps = psmm.tile([128, 512], F32, tag="mm")
                for ic in range(NIC):
                    nc.tensor.matmul(ps, vmov[:, ic, :],
                                     wvs[:, ic, :],
                                     start=(ic == 0), stop=(ic == NIC - 1))
                nc.vector.tensor_copy(vhs_own[:, j, :], ps)
                if j < 2:
                    continue
                # stage + all-gather chunk: j==2 -> blocks {0,1,2} (A),
                # j==3 -> block {3} (B)
                blo, bhi = CHUNK_BLKS[j - 2]
                nb = bhi - blo
                kv_cin = kv_cinA if j == 2 else kv_cinB
                kv_cout = kv_coutA if j == 2 else kv_coutB
                ksz = HKV * 128 * nb
                nc.sync.dma_start(
                    out=kv_cin[:, 0:ksz].rearrange("p (h m) -> p h m", h=HKV),
                    in_=khs_own[:, :, 128 * blo:128 * bhi])
                nc.sync.dma_start(
                    out=kv_cin[:, ksz:].rearrange("p (v m) -> p v m", v=nb),
                    in_=vhs_own[:, blo:bhi, :])
                nc.gpsimd.collective_compute(
                    "AllGather", mybir.AluOpType.bypass,
                    replica_groups=[[0, 1, 2, 3], [4, 5, 6, 7]],
                    ins=[kv_cin[:, :]], outs=[kv_cout[:, :, :]])
                # unstage: blocks {blo..bhi} of rank r = key tiles {4r+b}
                for r in range(4):
                    nc.sync.dma_start(
                        out=khs[:, :, 512 * r + 128 * blo:512 * r + 128 * bhi],
                        in_=kv_cout[r, :, 0:ksz].rearrange(
                            "p (h x) -> p h x", h=HKV))
                    nc.sync.dma_start(
                        out=vhs[:, 4 * r + blo:4 * r + bhi, :],
                        in_=kv_cout[r, :, ksz:].rearrange(
                            "p (v x) -> p v x", v=nb))

            # ---------------- Phase A: Q-proj + RoPE ----------------
            nc.sync.dma_start(out=qts, in_=qt[:, :].rearrange(
                "p (i m) -> p i m", i=NIC))
            cosq_t = res.tile([128, RPC], BF16)
            nc.sync.dma_start(out=cosq_t, in_=cosq[:, :])
            sinq_t = res.tile([128, RPC], BF16)
            nc.sync.dma_start(out=sinq_t, in_=sinq[:, :])
            for oc in range(H):
                wq_all = stream05.tile([128, NIC, 128], BF16, tag="s05")
                nc.sync.dma_start(out=wq_all, in_=wq[oc].rearrange(
                    "p (i m) -> p i m", i=NIC))
                ps = psmm.tile([128, 512], F32, tag="mm")
                for ic in range(NIC):
                    nc.tensor.matmul(ps, wq_all[:, ic, :],
                                     qts[:, ic, :],
                                     start=(ic == 0), stop=(ic == NIC - 1))
                xq = small.tile([128, 512], BF16, tag="xq")
                nc.scalar.copy(xq, ps)
                rope(qhs[:, oc, :], xq, psmm, cosq_t, sinq_t, RPC)

            # ---------------- Phase C: attention per head ----------------
            if mode == "mask":
                m01s = res.tile([128, NKC, RPC], BF16)
                nc.sync.dma_start(out=m01s, in_=m01[:, :].rearrange(
                    "p (k m) -> p k m", k=NKC))

            def normalize_batch(g):
                """reciprocal + broadcast + in-place normalize for the heads
                of batch g (their sums are already in sums_dram)."""
                a, bnd = NB[g]
                m = bnd - a
                nc.sync.dma_start(out=sums_g[g][:m, :],
                                  in_=sums_dram[a:bnd, :])
                nc.vector.reciprocal(rec_g[g][:m, :], sums_g[g][:m, :])
                nc.sync.dma_start(out=rec_dram[a:bnd, :], in_=rec_g[g][:m, :])
                for h in range(a, bnd):
                    recb = bcastp.tile([128, RPC], F32, tag="bc")
                    nc.sync.dma_start(
                        out=recb,
                        in_=rec_dram[h:h + 1, :].to_broadcast([128, RPC]))
                    nc.vector.tensor_mul(outu(h), outu(h), recb)

            LA = 1                # kc-step lookahead (software pipeline)

            # heads processed in pairs sharing a kv head: the K/V stationary
            # is identical for both, and for n <= 256 both heads' scores
            # pack into one PSUM bank so one ACT covers both exps.
            for hp in range(H // 2):
                h0, h1 = 2 * hp, 2 * hp + 1
                hk = h0 // G
                ps_o0 = psacc.tile([128, 512], F32, tag="acc", name="ps_o0")
                ps_o1 = psacc.tile([128, 512], F32, tag="acc", name="ps_o1")
                # both heads' row-sums in one bank: h0 on partition 0,
                # h1 on partition 32 (column-group tiling)
                ps_s = pssum.tile([128, 512], F32, tag="sum", name="ps_s")
                pending = {}

                def issue_scores(i, h0=h0, h1=h1, hk=hk, pending=pending):
                    kc = KC_ORDER[i]
                    n = n_of[kc]
                    lo = RPC - n          # suffix columns
                    kap = khs[:, hk, kc * 128:(kc + 1) * 128]
                    packed = 2 * n <= 512
                    if packed:
                        # one matmul computes both heads' scores: the moving
                        # operand is the 3D slice [128, 2 heads, n]
                        ps_sc = psmm.tile([128, 512], F32, tag="mm")
                        nc.tensor.matmul(ps_sc[:, :2 * n], kap,
                                         qhs[:, h0:h0 + 2, lo:],
                                         start=True, stop=True,
                                         skip_group_check=True)
                        probs = probsp.tile([128, 512], BF16, tag="pr")
                        nc.scalar.activation(
                            probs[:, :2 * n], ps_sc[:, :2 * n],
                            mybir.ActivationFunctionType.Exp, scale=SCALE)
                        pr0, pr1 = probs[:, :n], probs[:, n:2 * n]
                        d0, d1 = probs[:, 0:32], probs[:, n:n + 32]
                    else:
                        ps_a = psmm.tile([128, 512], F32, tag="mm")
                        nc.tensor.matmul(ps_a[:, :n], kap, qhs[:, h0, lo:],
                                         start=True, stop=True,
                                         skip_group_check=True)
                        ps_b = psmm.tile([128, 512], F32, tag="mm")
                        mm_dedup(ps_b[:, :n], kap, qhs[:, h1, lo:],
                                 start=True, stop=True,
                                 skip_group_check=True)
                        probs0 = probsp.tile([128, 512], BF16, tag="pr")
                        nc.scalar.activation(
                            probs0[:, :n], ps_a[:, :n],
                            mybir.ActivationFunctionType.Exp, scale=SCALE)
                        probs1 = probsp.tile([128, 512], BF16, tag="pr")
                        nc.scalar.activation(
                            probs1[:, :n], ps_b[:, :n],
                            mybir.ActivationFunctionType.Exp, scale=SCALE)
                        pr0, pr1 = probs0[:, :n], probs1[:, :n]
                        d0, d1 = probs0[:, 0:32], probs1[:, 0:32]
                    if mode == "causal":
                        # only the first 32 suffix columns (the diagonal
                        # 32-row q-block) are partially masked
                        nc.vector.tensor_mul(d0, d0, mdiag_t)
                        nc.vector.tensor_mul(d1, d1, mdiag_t)
                    elif mask_mul:
                        map_ = m01s[:, kc, lo:]
                        nc.vector.tensor_mul(pr0, pr0, map_)
                        nc.vector.tensor_mul(pr1, pr1, map_)
                    pending[i] = (pr0, pr1, kc, n, lo)

                for i in range(LA):
                    issue_scores(i)
                for idx in range(NKC):
                    if idx + LA < NKC:
                        issue_scores(idx + LA)
                    pr0, pr1, kc, n, lo = pending.pop(idx)
                    first = idx == 0
                    last = idx == NKC - 1
                    vap = vhs[:, kc, hk * 128:(hk + 1) * 128]
                    # the two row-sum matmuls target different column
                    # groups of the PE array and run concurrently
                    nc.tensor.matmul(ps_s[0:1, lo:], ones_t, pr0,
                                     start=first, stop=last,
                                     skip_group_check=True,
                                     tile_position=(0, 0))
                    nc.tensor.matmul(ps_s[32:33, lo:], ones_t, pr1,
                                     start=first, stop=last,
                                     skip_group_check=True,
                                     tile_position=(0, 32))
                    nc.tensor.matmul(ps_o0[:, lo:], vap, pr0,
                                     start=first, stop=last,
                                     skip_group_check=True)
                    mm_dedup(ps_o1[:, lo:], vap, pr1,
                             start=first, stop=last,
                             skip_group_check=True)
                for h, strip, ps_o in ((h0, 0, ps_o0), (h1, 32, ps_o1)):
                    sm1 = small.tile([1, RPC], F32, tag="sm1", bufs=2)
                    nc.vector.tensor_copy(sm1, ps_s[strip:strip + 1, :])
                    nc.sync.dma_start(out=sums_dram[h:h + 1, :], in_=sm1)
                    nc.vector.tensor_copy(outu(h), ps_o)
                if h1 == 7:
                    normalize_batch(0)
                elif h1 == 11:
                    normalize_batch(1)
            normalize_batch(2)

            # ---------------- Phase D: out-projection ----------------
            for oc in range(4):
                wo_all = stream2m.tile([128, H, 512], BF16, tag="s2m")
                nc.sync.dma_start(out=wo_all, in_=wo[oc].rearrange(
                    "p (h m) -> p h m", h=H))
                for qc in range(4):
                    ps_f = psmm.tile([128, 512], F32, tag="mm")
                    for h in range(H):
                        lh = outu_a[:, h, qc * 128:(qc + 1) * 128] if h < 12 \
                            else outu_b[:, h - 12, qc * 128:(qc + 1) * 128]
                        nc.tensor.matmul(
                            ps_f, lh, wo_all[:, h, :],
                            start=(h == 0), stop=(h == H - 1))
                    fin = small.tile([128, 512], F32, tag="fin")
                    nc.vector.tensor_copy(fin, ps_f)
                    nc.sync.dma_start(
                        out=out[qc * 128:(qc + 1) * 128,
                                oc * 512:(oc + 1) * 512],
                        in_=fin)

    nc.compile()
    return nc


def _get_nc(mode: str):
    if mode not in _NC_CACHE:
        _NC_CACHE[mode] = _build(mode)
    return _NC_CACHE[mode]


def _core_rows(mode: str, r: int) -> np.ndarray:
    """Global (within-batch) q-row indices owned by quarter r, ascending.

    causal: 16 interleaved 32-row blocks {4j + r : j} -> exact suffix
    causality, identical shapes on every core.  other modes: contiguous.
    """
    if mode == "causal":
        return np.concatenate([np.arange(32 * (4 * j + r), 32 * (4 * j + r + 1))
                               for j in range(16)])
    return np.arange(r * RPC, (r + 1) * RPC)


def kernel(q, k, v, mask, freqs, W_q, W_k, W_v, W_o):
    q = np.asarray(q, dtype=np.float32)
    k = np.asarray(k, dtype=np.float32)
    v = np.asarray(v, dtype=np.float32)
    mask = np.asarray(mask, dtype=np.float32)
    freqs = np.asarray(freqs, dtype=np.float32)
    W_q = np.asarray(W_q, dtype=np.float32)
    W_k = np.asarray(W_k, dtype=np.float32)
    W_v = np.asarray(W_v, dtype=np.float32)
    W_o = np.asarray(W_o, dtype=np.float32)

    # ---- mask mode detection ----
    nz = mask != 0
    if nz.all():
        mode = "none"
    else:
        tril = np.tril(np.ones((S, S), dtype=bool))
        mode = "causal" if all(np.array_equal(nz[b], tril) for b in range(B)) \
            else "mask"

    # ---- shared host precomputation ----
    c_full = np.cos(freqs)                      # [S, 64]
    s_full = np.sin(freqs)
    sgn = np.tile(np.array([-1.0, 1.0], np.float32), DH // 2)  # [-,+,-,+...]
    cosk_h = np.repeat(c_full, 2, axis=1).T.astype(BF)          # [128, S]
    sink_h = (np.repeat(s_full, 2, axis=1) * sgn).T.astype(BF)

    psw = np.zeros((128, 128), np.float32)
    idx = np.arange(128)
    psw[idx, idx ^ 1] = 1.0
    psw = psw.astype(BF)

    # weight layouts
    # wq[oc, p, i*128+m] = W_q[oc*128+m, i*128+p]
    wq_h = np.ascontiguousarray(
        W_q.reshape(H, 128, NIC, 128).transpose(0, 3, 2, 1)
        .reshape(H, 128, D)).astype(BF)
    # wk[hk, p, i*128+m] = W_k[hk*128+m, i*128+p]
    wk_h = np.ascontiguousarray(
        W_k.reshape(HKV, 128, NIC, 128).transpose(0, 3, 2, 1)
        .reshape(HKV, 128, D)).astype(BF)
    # wv[p, i*512+n] = W_v[n, i*128+p]
    wv_h = np.ascontiguousarray(
        W_v.reshape(DKV, NIC, 128).transpose(2, 1, 0).reshape(128, NIC * DKV)
    ).astype(BF)
    # wo[oc, p, h*512+m] = W_o[oc*512+m, h*128+p]
    wo_h = np.ascontiguousarray(
        W_o.reshape(4, 512, H, 128).transpose(0, 3, 2, 1).reshape(4, 128, -1)
    ).astype(BF)

    # k/v: each core only gets its own 512-token quarter (gathered on device)
    # kt[p, i*512+t] = k[b, tq*512+t, i*128+p] for quarter tq
    kt_b = []   # [B][4] quarters
    vt_b = []
    for b in range(B):
        kt_b.append([np.ascontiguousarray(
            k[b, tq * 512:(tq + 1) * 512].reshape(512, NIC, 128)
            .transpose(2, 1, 0).reshape(128, NIC * 512)).astype(BF)
            for tq in range(4)])
        # vt[j, p, i*128+t] = v[b, tq*512 + j*128+t, i*128+p]
        vt_b.append([np.ascontiguousarray(
            v[b, tq * 512:(tq + 1) * 512].reshape(4, 128, NIC, 128)
            .transpose(0, 3, 2, 1).reshape(4, 128, NIC * 128)).astype(BF)
            for tq in range(4)])

    in_maps = []
    rows_all = []
    for c in range(NCORES):
        b, r = divmod(c, 4)
        rows = _core_rows(mode, r)
        rows_all.append((b, rows))
        # qt[p, i*512+t] = q[b, rows[t], i*128+p]
        qsl = q[b][rows]                       # [512, D]
        qt_h = np.ascontiguousarray(
            qsl.reshape(RPC, NIC, 128).transpose(2, 1, 0).reshape(128, -1)
        ).astype(BF)
        cq = np.repeat(c_full[rows], 2, axis=1).T.astype(BF)      # [128, 512]
        sq = (np.repeat(s_full[rows], 2, axis=1) * sgn).T.astype(BF)
        im = {
            "wq": wq_h, "qt": qt_h, "kt": kt_b[b][r], "vt": vt_b[b][r],
            "wk": wk_h, "wv": wv_h, "wo": wo_h,
            "cosq": cq, "sinq": sq,
            "cosk": np.ascontiguousarray(cosk_h[:, r * 512:(r + 1) * 512]),
            "sink": np.ascontiguousarray(sink_h[:, r * 512:(r + 1) * 512]),
            "pswap": psw,
        }
        if mode == "causal":
            # diagonal 32-col block mask: keep key p of tile kc for local
            # row i of block j=kc  <=>  p <= 32*r + i  (same for all kc)
            pp = np.arange(128)[:, None]
            ii = np.arange(32)[None, :]
            im["mdiag"] = (pp <= 32 * r + ii).astype(BF)
        elif mode == "mask":
            # m01[p, kc*512+m] = (mask[b, rows[m], kc*128+p] != 0)
            msl = nz[b][rows]                  # [512, S] bool
            m01_h = np.ascontiguousarray(
                msl.T.reshape(NKC, 128, RPC).transpose(1, 0, 2)
                .reshape(128, -1)).astype(BF)
            im["m01"] = m01_h
        in_maps.append(im)

    nc = _get_nc(mode)
    kwargs = {}
    if TRACE:
        kwargs["trace"] = True
        if TRACE_CORES:
            kwargs["trace_cores"] = list(TRACE_CORES)
    results = run_bass_kernel_spmd(nc, in_maps, core_ids=list(range(NCORES)),
                                   **kwargs)
    global LAST_RESULTS
    LAST_RESULTS = results

    full = np.empty((B, S, D), np.float32)
    for c in range(NCORES):
        b, rows = rows_all[c]
        full[b, rows] = results.results[c]["out"]
    return full
